# revision 32
# baseline (speedup 1.0000x reference)
"""Trainium2 Bass kernel for nn_CrossAttention (B=4, NQ=NK=1024, D=1024, H=16).

Sharding: 8 cores = 4 batches x 2 head-groups (8 heads each). Per core:
  - inputs arrive pre-transposed/sliced on host (free): xqT/xkT/xvT [D, T] fp16,
    Wq/Wk/Wv column slices [D, 512] fp16, Wo row slice [512, D] fp16.
  - warm-up matmuls on a zeroed scratch tile run from t~7us (during the DMA
    fill) so the PE HAM clock-gate is released before real matmuls start.
  - input DMAs spread over 4 hw queues (gpsimd/sync/vector/scalar) with pair-0
    weights at the head of the gpsimd queue and xq/xk each split even/odd
    across two queues, so the projection k-frontier is never DMA-starved.
  - projections produce Q^T/K^T per head-pair [128, T] (lhsT = W slice, rhs =
    xT) in 2-chain groups (shared LDWEIGHTS) on a 2-slot PSUM ring.
  - scores computed transposed (scoresT [Tk, Tq]) as 4-way tile_position-packed
    quads (2 heads x 2 M-halves, K=64 each) into 2-bank PSUM tiles (3-deep
    ring); ONE fused exp activation per quad (free size 1024).
  - the exp stream starts at ~21us and must never starve: 40 early quads
    (all of chunk 0 + pair 0 of chunk 1) are interleaved into the projection
    chains, the rest follow with +5 lookahead in the PV loop.
  - denominators via an augmented ones-column in V (row 64 of the PV output);
    reciprocal via 2-op approx on [2,512]; the per-query reciprocals are
    broadcast to 128 partitions with ONE K=2 PE matmul (selector lhsT), so
    gpsimd is off the critical path entirely.
  - per query-chunk out-projection overlaps the next chunk's scores/PV; final
    chunk accumulates p4-major across 8 PSUM banks, copies alternate
    scalar/vector, output DMAs alternate sync/gpsimd.
  - host sums the two head-group partials per batch and adds the bias.
All matmuls fp16 (1 cycle/row on PE), accumulation fp32 in PSUM.
"""
import sys

sys.path.insert(0, "/opt/trn_rl_repo")

from contextlib import ExitStack

import numpy as np

import concourse.bass as bass
import concourse.tile as tile
from concourse import bacc, mybir
from concourse.bass_utils import run_bass_kernel_spmd

F32 = mybir.dt.float32
F16 = mybir.dt.float16

B, NQ, NK, D, H, HD = 4, 1024, 1024, 1024, 16, 64
NCORES = 8
HPC = 8          # heads per core
F = HPC * HD     # 512: per-core projection width
KT = D // 128    # 8 k-tiles over D
PAIRS = HPC // 2  # 4 head pairs
TKT = NK // 128  # 8 tiles over key tokens
NCH = NQ // 512  # 2 query chunks

N_WARM = 9       # warm-up matmuls (~3.8us cold) to release the HAM throttle
H1_DIRECT = True  # DVE mul writing partitions 64-127 from inputs at 0-63


def _emit(tc):
    nc = tc.nc
    ctx = ExitStack()

    xqT = nc.dram_tensor("xqT", [D, NQ], F16, kind="ExternalInput").ap()
    xkT = nc.dram_tensor("xkT", [D, NK], F16, kind="ExternalInput").ap()
    xvT = nc.dram_tensor("xvT", [D, NK], F16, kind="ExternalInput").ap()
    # wq/wk host-swizzled to [m-pair][sbuf-partition][k-tile*128]: one
    # contiguous 256KB DMA per pair
    wq = nc.dram_tensor("wq", [PAIRS * 128, D], F16, kind="ExternalInput").ap()
    wk = nc.dram_tensor("wk", [PAIRS * 128, D], F16, kind="ExternalInput").ap()
    wv = nc.dram_tensor("wv", [D, F], F16, kind="ExternalInput").ap()
    wo = nc.dram_tensor("wo", [F, D], F16, kind="ExternalInput").ap()
    out = nc.dram_tensor("out", [NQ, D], F16, kind="ExternalOutput").ap()

    wpool = ctx.enter_context(tc.tile_pool(name="wpool", bufs=1))
    qkv = ctx.enter_context(tc.tile_pool(name="qkv", bufs=1))
    xpool = ctx.enter_context(tc.tile_pool(name="xpool", bufs=16))
    expool = ctx.enter_context(tc.tile_pool(name="expool", bufs=42))
    psum = ctx.enter_context(tc.tile_pool(name="psum", bufs=1, space="PSUM"))
    nrm = ctx.enter_context(tc.tile_pool(name="nrm", bufs=2))
    ost = ctx.enter_context(tc.tile_pool(name="ost", bufs=4))

    # zero-bias AP for activations (avoids the const-page TENSOR_LOAD)
    zb = nrm.tile([128, 1], F32, tag="zb", bufs=1)
    nc.vector.memset(zb[:], 0.0)
    # ones selector for the K=1 reciprocal-broadcast matmuls
    selA = nrm.tile([1, 64], F16, tag="selA", bufs=1)
    nc.vector.memset(selA[:], 1.0)
    # warm-up scratch (memset on gpsimd: ready before the vector preamble)
    warm = nrm.tile([128, 512], F16, tag="warm", bufs=1)
    nc.gpsimd.memset(warm[:], 0.0)
    rscr2 = nrm.tile([1, 1024], F32, tag="rscr2", bufs=1)
    # preload the exp table set during the DMA fill (saves ~2.7us at the
    # first real exp)
    tload = nrm.tile([128, 1], F16, tag="tload", bufs=1)
    nc.scalar.activation(out=tload[:], in_=zb[:],
                         func=mybir.ActivationFunctionType.Exp,
                         scale=1.0, bias=zb[:])

    # ---- warm-up matmuls: PE busy from t~7.3us so HAM hits K=8/8 by the time
    # the first real projection matmul runs
    for i in range(N_WARM):
        wps = psum.tile([128, 512], F32, tag="ps", bufs=2, name=f"warm{i}")
        nc.tensor.matmul(out=wps[:], lhsT=warm[:, 0:128], rhs=warm[:],
                         start=True, stop=True)

    # ---- input DMAs over 3 queues. x tiles get the gpsimd AND sync queues
    # (even/odd split per tensor); weights ride the scalar queue with pairs
    # 2/3 in a 2-slot ring so their transfers self-gate (WAR semaphore)
    # behind pair 0/1's projection reads instead of crowding the x stream.
    wq_t = [wpool.tile([128, KT, 128], F16, tag="wqr", bufs=2, name=f"wqr{m}")
            for m in range(PAIRS)]
    wk_t = [wpool.tile([128, KT, 128], F16, tag="wkr", bufs=2, name=f"wkr{m}")
            for m in range(PAIRS)]
    wv_sb = wpool.tile([128, KT, F], F16, tag="wv")
    wo_sb = wpool.tile([128, PAIRS, D], F16, tag="wo")

    xq_t, xk_t, xv_t = [], [], []
    for k in range(KT):
        xq_t.append(xpool.tile([128, NQ], F16, tag="x", name=f"xq{k}"))
        xk_t.append(xpool.tile([128, NK], F16, tag="x", name=f"xk{k}"))

    # gpsimd queue: xq evens, xk evens, wv (x tiles lead: the queue is
    # in-order, so anything ahead of xq delays the first projection)
    for k in range(0, KT, 2):
        nc.gpsimd.dma_start(out=xq_t[k][:], in_=xqT[k * 128:(k + 1) * 128, :])
    for k in range(0, KT, 2):
        nc.gpsimd.dma_start(out=xk_t[k][:], in_=xkT[k * 128:(k + 1) * 128, :])
    for k in range(KT):
        nc.gpsimd.dma_start(out=wv_sb[:, k, :], in_=wv[k * 128:(k + 1) * 128, :])
    # sync queue: xq odds, xk odds (xv follows after the QK loop: its tiles
    # reuse the xq/xk ring slots, so the writes must be emitted after reads)
    for k in range(1, KT, 2):
        nc.sync.dma_start(out=xq_t[k][:], in_=xqT[k * 128:(k + 1) * 128, :])
    for k in range(1, KT, 2):
        nc.sync.dma_start(out=xk_t[k][:], in_=xkT[k * 128:(k + 1) * 128, :])
    # scalar queue: QK weights only (pairs 2/3 ring-gated); wo rides the END
    # of the sync queue (behind the WAR-gated xv) so its 1MB never competes
    # with the phase-1 x stream
    for m in range(PAIRS):
        nc.scalar.dma_start(out=wq_t[m][:], in_=wq[m * 128:(m + 1) * 128, :])
        nc.scalar.dma_start(out=wk_t[m][:], in_=wk[m * 128:(m + 1) * 128, :])

    # ---- persistent intermediates ----
    qt = [qkv.tile([128, NQ], F16, tag=f"qt{p}", name=f"qt{p}") for p in range(PAIRS)]
    kt = [qkv.tile([128, NK], F16, tag=f"kt{p}", name=f"kt{p}") for p in range(PAIRS)]
    vp_sb = qkv.tile([128, TKT, HPC, HD + 1], F16, tag="vp")  # V + ones col
    att = [qkv.tile([128, NQ], F16, tag=f"att{p}", name=f"att{p}") for p in range(PAIRS)]
    nc.vector.memset(vp_sb[:, :, :, HD:HD + 1], 1.0)

    scale = 1.0 / float(np.sqrt(HD))
    ex = {}

    def emit_quad(p, c, tkm):
        """One scoresT quad (2 heads x 128 keys x 512 queries) + fused exp."""
        ps = psum.tile([128, 2, 512], F32, tag="quad", bufs=2,
                       name=f"qps_{p}_{c}_{tkm}")
        et = expool.tile([128, 2, 512], F16, tag="ex", name=f"ex_{p}_{c}_{tkm}")
        for hh in range(2):
            r0 = hh * 64
            for mh in range(2):
                c0 = mh * 64
                nc.tensor.matmul(
                    out=ps[c0:c0 + 64, hh, :],
                    lhsT=kt[p][r0:r0 + 64, tkm * 128 + c0:tkm * 128 + c0 + 64],
                    rhs=qt[p][r0:r0 + 64, c * 512:(c + 1) * 512],
                    start=True, stop=True,
                    tile_position=(r0, c0))
        nc.scalar.activation(out=et[:], in_=ps[:],
                             func=mybir.ActivationFunctionType.Exp,
                             scale=scale, bias=zb[:])
        ex[(p, c, tkm)] = et

    # early quads: all of chunk 0 plus pair 0 of chunk 1, interleaved into the
    # projection chains so the exp stream runs nonstop from ~22us; chunk-0
    # pairs first so each PV iteration's exps complete as early as possible
    early = [(p, 0, k) for p in range(PAIRS) for k in range(TKT)]
    early += [(0, 1, k) for k in range(TKT)]
    early.sort(key=lambda t: (t[1], t[0]))
    ei = 0

    def pop_quads(n, ready_pairs):
        nonlocal ei
        while n > 0 and ei < len(early):
            p, c, k = early[ei]
            if p >= ready_pairs:
                return
            emit_quad(p, c, k)
            ei += 1
            n -= 1

    # ---- QK projections: 2-chain groups (both 512-query chunks of one
    # tensor) sharing LDWEIGHTS, k-outer so the PE tracks the DMA frontier.
    # Group order (0,q),(1,q),(0,k),(1,k),(2,q),(2,k),(3,q),(3,k): pair 1's
    # q-chains (xq + ungated wq1 only) fill pair 0's xk DMA window with real
    # work, so the PE never idles long enough to re-throttle.
    ready = 0
    for m, gi in ((0, 0), (1, 0), (0, 1), (1, 1),
                  (2, 0), (2, 1), (3, 0), (3, 1)):
        x_t, w_sb, dst = ((xq_t, wq_t[m], qt[m]) if gi == 0 else
                          (xk_t, wk_t[m], kt[m]))
        # quads of completed pairs between chain groups keep the exp
        # stream fed while this pair's projections accumulate
        pop_quads(2 if ready else 0, ready_pairs=ready)
        pp = [psum.tile([128, 512], F32, tag="ps", bufs=2,
                        name=f"ps_p{m}_{gi}_{n}") for n in range(NCH)]
        for k in range(KT):
            if k == 4:
                pop_quads(1, ready_pairs=ready)
            for n in range(NCH):
                nc.tensor.matmul(out=pp[n][:],
                                 lhsT=w_sb[:, k, :],
                                 rhs=x_t[k][:, n * 512:(n + 1) * 512],
                                 start=(k == 0), stop=(k == KT - 1))
        for n in range(NCH):
            nc.vector.tensor_copy(out=dst[:, n * 512:(n + 1) * 512],
                                  in_=pp[n][:])
        if gi == 1:
            # pair m's qt/kt both complete: its quads become poppable
            ready = m + 1
            pop_quads(3, ready_pairs=ready)

    # xv DMAs (sync queue): the tiles reuse the xq ring slots, released as
    # pair 3's q-chains finish reading each k-tile
    for k in range(KT):
        xv_t.append(xpool.tile([128, NK], F16, tag="x", name=f"xv{k}"))
        nc.sync.dma_start(out=xv_t[k][:], in_=xvT[k * 128:(k + 1) * 128, :])
    for p in range(PAIRS):
        nc.sync.dma_start(out=wo_sb[:, p, :], in_=wo[p * 128:(p + 1) * 128, :])

    # ---- V projection: 4 groups of 2 token-chains on the 2-slot ps ring ----
    for g in range(4):
        chains = []
        for t in range(g * 2, g * 2 + 2):
            psv = psum.tile([128, 512], F32, tag="ps", bufs=2, name=f"psv_{t}")
            chains.append((psv, t))
        for k in range(KT):
            if k in (3, 6):
                pop_quads(1, ready_pairs=PAIRS)
            for psv, t in chains:
                nc.tensor.matmul(out=psv[:],
                                 lhsT=xv_t[k][:, t * 128:(t + 1) * 128],
                                 rhs=wv_sb[:, k, :], start=(k == 0),
                                 stop=(k == KT - 1))
        for psv, t in chains:
            nc.vector.tensor_copy(
                out=vp_sb[:, t, :, 0:HD],
                in_=psv[:].rearrange("p (h d) -> p h d", h=HPC))
            pop_quads(3, ready_pairs=PAIRS)
    # any leftover early quads
    pop_quads(len(early), ready_pairs=PAIRS)

    def norm_dve(p, c, pvq):
        """DVE half of the softmax normalization: PV copies (releasing the
        pvq banks), denominator row, reciprocal, f16 cast."""
        pvsb = nrm.tile([65, 2, 512], F16, tag="pvsb", name=f"pvsb_{p}_{c}")
        nc.vector.tensor_copy(out=pvsb[:, 0, :], in_=pvq[0:65, 0, :])
        nc.vector.tensor_copy(out=pvsb[:, 1, :], in_=pvq[0:65, 1, :])
        den2 = nrm.tile([1, 1024], F32, tag="den2", name=f"den2_{p}_{c}")
        nc.vector.tensor_copy(
            out=den2[:].rearrange("p (h q) -> p h q", h=2),
            in_=pvsb[64:65, :, :])
        rec2 = nrm.tile([1, 1024], F32, tag="rec2", name=f"rec2_{p}_{c}")
        nc.vector.reciprocal_approx_accurate(out=rec2[:], in_=den2[:],
                                             scratch=rscr2[:])
        rec2h = nrm.tile([1, 1024], F16, tag="rec2h", name=f"rec2h_{p}_{c}")
        nc.vector.tensor_copy(out=rec2h[:], in_=rec2[:])
        return (p, c, pvsb, rec2h)

    def norm_pe(pend):
        """PE half, applied one iteration later so the broadcast matmuls
        never wait on the DVE chain at the head of the PE queue: two
        concurrent col-tiled K=1 broadcasts, then the normalizing muls."""
        p, c, pvsb, rec2h = pend
        cs = slice(c * 512, (c + 1) * 512)
        rb = psum.tile([128, 512], F32, tag="ps", bufs=2, name=f"rb_{p}_{c}")
        nc.tensor.matmul(out=rb[0:64, :], lhsT=selA[:], rhs=rec2h[:, 0:512],
                         start=True, stop=True, tile_position=(0, 0))
        nc.tensor.matmul(out=rb[64:128, :], lhsT=selA[:], rhs=rec2h[:, 512:1024],
                         start=True, stop=True, tile_position=(0, 64))
        nc.vector.tensor_mul(out=att[p][0:64, cs], in0=pvsb[0:64, 0, :],
                             in1=rb[0:64, :])
        nc.vector.tensor_mul(out=att[p][64:128, cs], in0=pvsb[0:64, 1, :],
                             in1=rb[64:128, :])

    tiles_w = [(4 + qi, half) for qi in range(4) for half in range(2)]
    psos_w = []

    def emit_out_wide_pre():
        """Final-chunk out-projection, phase 1: ranks p4=0..2 for 4 output
        tiles in the quad pool — emitted BEFORE the last normalization so
        they aren't queued behind its broadcast matmuls."""
        for j in range(2):
            qa = psum.tile([128, 2, 512], F32, tag="quad", bufs=2,
                           name=f"oq_{j}")
            psos_w.extend([qa[:, 0, :], qa[:, 1, :]])
        for p4 in range(PAIRS - 1):
            for ti in range(4):
                q, half = tiles_w[ti]
                nc.tensor.matmul(out=psos_w[ti],
                                 lhsT=att[p4][:, q * 128:(q + 1) * 128],
                                 rhs=wo_sb[:, p4, half * 512:(half + 1) * 512],
                                 start=(p4 == 0), stop=False)

    def emit_out_wide_post():
        """Phase 2 (after the last norm's allocations): the pv slot + 2
        ps-ring tiles for tiles 4..7, then rank 3 everywhere, then copies."""
        ov = psum.tile([128, 2, 512], F32, tag="pv", bufs=1, name="ov")
        psos_w.extend([ov[:, 0, :], ov[:, 1, :]])
        for j in range(2):
            psos_w.append(psum.tile([128, 512], F32, tag="ps", bufs=2,
                                    name=f"os_{j}"))
        for p4 in range(PAIRS - 1):
            for ti in (4, 5, 6, 7):
                q, half = tiles_w[ti]
                nc.tensor.matmul(out=psos_w[ti],
                                 lhsT=att[p4][:, q * 128:(q + 1) * 128],
                                 rhs=wo_sb[:, p4, half * 512:(half + 1) * 512],
                                 start=(p4 == 0), stop=False)
        for ti in range(8):
            q, half = tiles_w[ti]
            nc.tensor.matmul(out=psos_w[ti],
                             lhsT=att[PAIRS - 1][:, q * 128:(q + 1) * 128],
                             rhs=wo_sb[:, PAIRS - 1, half * 512:(half + 1) * 512],
                             start=False, stop=True)
        for i, ((q, half), pso) in enumerate(zip(tiles_w, psos_w)):
            ot = ost.tile([128, 512], F16, tag="ot", name=f"ot_{q}_{half}")
            # exp stream is drained here: split copies scalar/vector
            if i % 2 == 0:
                nc.scalar.copy(out=ot[:], in_=pso)
            else:
                nc.vector.tensor_copy(out=ot[:], in_=pso)
            eng = nc.sync if i % 2 == 0 else nc.gpsimd
            eng.dma_start(out=out[q * 128:(q + 1) * 128,
                                  half * 512:(half + 1) * 512], in_=ot[:])

    tiles0 = [(qi, half) for qi in range(4) for half in range(2)]

    def emit_out0_part(ti_list):
        """2-3 chunk-0 out-projection tiles on the ps ring; spread across
        iterations 4-6 so their ring stalls hide in the exp-paced slack."""
        for j, ti in enumerate(ti_list):
            q, half = tiles0[ti]
            pso = psum.tile([128, 512], F32, tag="ps", bufs=2,
                            name=f"pso_{q}_{half}")
            for p4 in range(PAIRS):
                nc.tensor.matmul(out=pso[:],
                                 lhsT=att[p4][:, q * 128:(q + 1) * 128],
                                 rhs=wo_sb[:, p4, half * 512:(half + 1) * 512],
                                 start=(p4 == 0), stop=(p4 == PAIRS - 1))
            ot = ost.tile([128, 512], F16, tag="ot", name=f"ot_{q}_{half}")
            nc.vector.tensor_copy(out=ot[:], in_=pso[:])
            eng = nc.sync if ti % 2 == 0 else nc.gpsimd
            eng.dma_start(out=out[q * 128:(q + 1) * 128,
                                  half * 512:(half + 1) * 512], in_=ot[:])

    # ---- pipelined (pair, chunk) iterations ----
    # PV pairs write a dedicated 2-bank PSUM tile. Emission order per
    # iteration: PV, the PREVIOUS iteration's norm-PE half (its DVE inputs
    # are ready by now, so the broadcast matmuls never stall the PE queue),
    # quads for seq[i+5], out-proj blocks, then this iteration's norm-DVE.
    seq = [(p, c) for c in range(NCH) for p in range(PAIRS)]
    pending = None
    for i, (p, c) in enumerate(seq):
        pvq = psum.tile([128, 2, 512], F32, tag="pv", bufs=1,
                        name=f"pv_{p}_{c}")
        for k in range(TKT):
            et = ex[(p, c, k)]
            for hh in range(2):
                h = p * 2 + hh
                nc.tensor.matmul(out=pvq[0:65, hh, :],
                                 lhsT=vp_sb[:, k, h, :],
                                 rhs=et[:, hh, :],
                                 start=(k == 0), stop=(k == TKT - 1))
            del ex[(p, c, k)]
        if pending is not None:
            norm_pe(pending)
        if i + 5 < len(seq):
            for k in range(TKT):
                emit_quad(seq[i + 5][0], seq[i + 5][1], k)
        if i == 4:
            emit_out0_part([0, 1, 2])
        elif i == 5:
            emit_out0_part([3, 4, 5])
        elif i == 6:
            emit_out0_part([6, 7])
        if i == len(seq) - 1:
            emit_out_wide_pre()
        pending = norm_dve(p, c, pvq)
    norm_pe(pending)
    emit_out_wide_post()
    ctx.close()


_NC_CACHE = None


def build():
    global _NC_CACHE
    if _NC_CACHE is None:
        nc = bacc.Bacc("TRN2", target_bir_lowering=False, debug=False,
                       num_devices=NCORES)
        with tile.TileContext(nc) as tc:
            _emit(tc)
        nc.compile()
        _NC_CACHE = nc
    return _NC_CACHE


def make_in_maps(inputs):
    q = np.asarray(inputs["query_tokens"], dtype=np.float32)
    kk = np.asarray(inputs["key_tokens"], dtype=np.float32)
    v = np.asarray(inputs["value_tokens"], dtype=np.float32)
    Wq = np.asarray(inputs["Wq"], dtype=np.float32)
    Wk = np.asarray(inputs["Wk"], dtype=np.float32)
    Wv = np.asarray(inputs["Wv"], dtype=np.float32)
    Wo = np.asarray(inputs["Wo"], dtype=np.float32)

    def swizzle(w_cols):
        # [1024, 512] -> [pair m][sbuf partition p][k-tile k][d]: block m is a
        # contiguous [128, 1024] so one DMA per pair lands pair-major
        a = w_cols.reshape(8, 128, 4, 128).transpose(2, 1, 0, 3)
        return np.ascontiguousarray(a.reshape(512, 1024)).astype(np.float16)

    qT = [np.ascontiguousarray(q[b].T).astype(np.float16) for b in range(B)]
    kT = [np.ascontiguousarray(kk[b].T).astype(np.float16) for b in range(B)]
    vT = [np.ascontiguousarray(v[b].T).astype(np.float16) for b in range(B)]
    wq_g = [swizzle(Wq[:, g * F:(g + 1) * F]) for g in range(2)]
    wk_g = [swizzle(Wk[:, g * F:(g + 1) * F]) for g in range(2)]
    wv_g = [np.ascontiguousarray(Wv[:, g * F:(g + 1) * F]).astype(np.float16)
            for g in range(2)]
    wo_g = [np.ascontiguousarray(Wo[g * F:(g + 1) * F, :]).astype(np.float16)
            for g in range(2)]

    in_maps = []
    for c in range(NCORES):
        b, g = c // 2, c % 2
        in_maps.append({
            "xqT": qT[b], "xkT": kT[b], "xvT": vT[b],
            "wq": wq_g[g], "wk": wk_g[g], "wv": wv_g[g], "wo": wo_g[g],
        })
    return in_maps


def combine(results, bo):
    out = np.zeros((B, NQ, D), dtype=np.float32)
    for c in range(NCORES):
        out[c // 2] += results[c]["out"].astype(np.float32)
    out += np.asarray(bo, dtype=np.float32)[None, None, :]
    return out


def kernel(**inputs):
    nc = build()
    in_maps = make_in_maps(inputs)
    res = run_bass_kernel_spmd(nc, in_maps, list(range(NCORES)))
    return combine(res.results, inputs["bo"])


# revision 34
# speedup vs baseline: 1.1582x; 1.1582x over previous
"""Trainium2 Bass kernel for nn_CrossAttention (B=4, NQ=NK=1024, D=1024, H=16).

Sharding: 8 cores = 4 batches x 2 head-groups (8 heads each). Per core:
  - inputs arrive pre-transposed/sliced on host (free): xqT/xkT/xvT [D, T] fp16,
    Wq/Wk/Wv column slices [D, 512] fp16, Wo row slice [512, D] fp16.
  - warm-up matmuls on a zeroed scratch tile run from t~7us (during the DMA
    fill) so the PE HAM clock-gate is released before real matmuls start.
  - input DMAs spread over 4 hw queues (gpsimd/sync/vector/scalar) with pair-0
    weights at the head of the gpsimd queue and xq/xk each split even/odd
    across two queues, so the projection k-frontier is never DMA-starved.
  - projections produce Q^T/K^T per head-pair [128, T] (lhsT = W slice, rhs =
    xT) in 2-chain groups (shared LDWEIGHTS) on a 2-slot PSUM ring.
  - scores computed transposed (scoresT [Tk, Tq]) as 4-way tile_position-packed
    quads (2 heads x 2 M-halves, K=64 each) into 2-bank PSUM tiles (3-deep
    ring); ONE fused exp activation per quad (free size 1024).
  - the exp stream starts at ~21us and must never starve: 40 early quads
    (all of chunk 0 + pair 0 of chunk 1) are interleaved into the projection
    chains, the rest follow with +5 lookahead in the PV loop.
  - denominators via an augmented ones-column in V (row 64 of the PV output);
    reciprocal via 2-op approx on [2,512]; the per-query reciprocals are
    broadcast to 128 partitions with ONE K=2 PE matmul (selector lhsT), so
    gpsimd is off the critical path entirely.
  - per query-chunk out-projection overlaps the next chunk's scores/PV; final
    chunk accumulates p4-major across 8 PSUM banks, copies alternate
    scalar/vector, output DMAs alternate sync/gpsimd.
  - host sums the two head-group partials per batch and adds the bias.
All matmuls fp16 (1 cycle/row on PE), accumulation fp32 in PSUM.
"""
import sys

sys.path.insert(0, "/opt/trn_rl_repo")

from contextlib import ExitStack

import numpy as np

import concourse.bass as bass
import concourse.tile as tile
from concourse import bacc, mybir
from concourse.bass_utils import run_bass_kernel_spmd

F32 = mybir.dt.float32
F16 = mybir.dt.float16

B, NQ, NK, D, H, HD = 4, 1024, 1024, 1024, 16, 64
NCORES = 8
HPC = 8          # heads per core
F = HPC * HD     # 512: per-core projection width
KT = D // 128    # 8 k-tiles over D
PAIRS = HPC // 2  # 4 head pairs
TKT = NK // 128  # 8 tiles over key tokens
NCH = NQ // 512  # 2 query chunks

N_WARM = 9       # warm-up matmuls (~3.8us cold) to release the HAM throttle
H1_DIRECT = True  # DVE mul writing partitions 64-127 from inputs at 0-63


def _emit(tc):
    nc = tc.nc
    ctx = ExitStack()

    xqT = nc.dram_tensor("xqT", [D, NQ], F16, kind="ExternalInput").ap()
    xkT = nc.dram_tensor("xkT", [D, NK], F16, kind="ExternalInput").ap()
    xvT = nc.dram_tensor("xvT", [D, NK], F16, kind="ExternalInput").ap()
    # wq/wk host-swizzled to [m-pair][sbuf-partition][k-tile*128]: one
    # contiguous 256KB DMA per pair
    wq = nc.dram_tensor("wq", [PAIRS * 128, D], F16, kind="ExternalInput").ap()
    wk = nc.dram_tensor("wk", [PAIRS * 128, D], F16, kind="ExternalInput").ap()
    wv = nc.dram_tensor("wv", [D, F], F16, kind="ExternalInput").ap()
    wo = nc.dram_tensor("wo", [F, D], F16, kind="ExternalInput").ap()
    out = nc.dram_tensor("out", [NQ, D], F16, kind="ExternalOutput").ap()

    wpool = ctx.enter_context(tc.tile_pool(name="wpool", bufs=1))
    qkv = ctx.enter_context(tc.tile_pool(name="qkv", bufs=1))
    xpool = ctx.enter_context(tc.tile_pool(name="xpool", bufs=16))
    expool = ctx.enter_context(tc.tile_pool(name="expool", bufs=42))
    psum = ctx.enter_context(tc.tile_pool(name="psum", bufs=1, space="PSUM"))
    nrm = ctx.enter_context(tc.tile_pool(name="nrm", bufs=2))
    ost = ctx.enter_context(tc.tile_pool(name="ost", bufs=4))

    # zero-bias AP for activations (avoids the const-page TENSOR_LOAD)
    zb = nrm.tile([128, 1], F32, tag="zb", bufs=1)
    nc.vector.memset(zb[:], 0.0)
    # ones selector for the K=1 reciprocal-broadcast matmuls
    selA = nrm.tile([1, 64], F16, tag="selA", bufs=1)
    nc.vector.memset(selA[:], 1.0)
    # warm-up scratch (memset on gpsimd: ready before the vector preamble)
    warm = nrm.tile([128, 512], F16, tag="warm", bufs=1)
    nc.gpsimd.memset(warm[:], 0.0)
    rscr2 = nrm.tile([1, 1024], F32, tag="rscr2", bufs=1)
    # preload the exp table set during the DMA fill (saves ~2.7us at the
    # first real exp)
    tload = nrm.tile([128, 1], F16, tag="tload", bufs=1)
    nc.scalar.activation(out=tload[:], in_=zb[:],
                         func=mybir.ActivationFunctionType.Exp,
                         scale=1.0, bias=zb[:])

    # ---- warm-up matmuls: PE busy from t~7.3us so HAM hits K=8/8 by the time
    # the first real projection matmul runs
    for i in range(N_WARM):
        wps = psum.tile([128, 512], F32, tag="ps", bufs=2, name=f"warm{i}")
        nc.tensor.matmul(out=wps[:], lhsT=warm[:, 0:128], rhs=warm[:],
                         start=True, stop=True)

    # ---- input DMAs over 3 queues. x tiles get the gpsimd AND sync queues
    # (even/odd split per tensor); weights ride the scalar queue with pairs
    # 2/3 in a 2-slot ring so their transfers self-gate (WAR semaphore)
    # behind pair 0/1's projection reads instead of crowding the x stream.
    wq_t = [wpool.tile([128, KT, 128], F16, tag="wqr", bufs=2, name=f"wqr{m}")
            for m in range(PAIRS)]
    wk_t = [wpool.tile([128, KT, 128], F16, tag="wkr", bufs=2, name=f"wkr{m}")
            for m in range(PAIRS)]
    wv_sb = wpool.tile([128, KT, F], F16, tag="wv")
    wo_sb = wpool.tile([128, PAIRS, D], F16, tag="wo")

    xq_t, xk_t, xv_t = [], [], []
    for k in range(KT):
        xq_t.append(xpool.tile([128, NQ], F16, tag="x", name=f"xq{k}"))
        xk_t.append(xpool.tile([128, NK], F16, tag="x", name=f"xk{k}"))

    # gpsimd queue: wq0, wq1 (512KB, needed in the first ~18us), xq evens,
    # xk evens, wv. The queue is in-order, so the critical-path order is
    # exactly the emission order.
    nc.gpsimd.dma_start(out=wq_t[0][:], in_=wq[0:128, :])
    nc.gpsimd.dma_start(out=wq_t[1][:], in_=wq[128:256, :])
    for k in range(0, KT, 2):
        nc.gpsimd.dma_start(out=xq_t[k][:], in_=xqT[k * 128:(k + 1) * 128, :])
    for k in range(0, KT, 2):
        nc.gpsimd.dma_start(out=xk_t[k][:], in_=xkT[k * 128:(k + 1) * 128, :])
    for k in range(KT):
        nc.gpsimd.dma_start(out=wv_sb[:, k, :], in_=wv[k * 128:(k + 1) * 128, :])
    # sync queue: wk0, wk1, xq odds, xk odds (xv + wo follow after the QK
    # loop: xv tiles reuse the xq/xk ring slots)
    nc.sync.dma_start(out=wk_t[0][:], in_=wk[0:128, :])
    nc.sync.dma_start(out=wk_t[1][:], in_=wk[128:256, :])
    for k in range(1, KT, 2):
        nc.sync.dma_start(out=xq_t[k][:], in_=xqT[k * 128:(k + 1) * 128, :])
    for k in range(1, KT, 2):
        nc.sync.dma_start(out=xk_t[k][:], in_=xkT[k * 128:(k + 1) * 128, :])
    # scalar queue: only the ring-gated pair 2/3 weights — their descriptors
    # self-gate (WAR) behind pair 0/1's reads, so this queue never competes
    # with the phase-1 x stream
    for m in range(2, PAIRS):
        nc.scalar.dma_start(out=wq_t[m][:], in_=wq[m * 128:(m + 1) * 128, :])
        nc.scalar.dma_start(out=wk_t[m][:], in_=wk[m * 128:(m + 1) * 128, :])

    # ---- persistent intermediates ----
    qt = [qkv.tile([128, NQ], F16, tag=f"qt{p}", name=f"qt{p}") for p in range(PAIRS)]
    kt = [qkv.tile([128, NK], F16, tag=f"kt{p}", name=f"kt{p}") for p in range(PAIRS)]
    vp_sb = qkv.tile([128, TKT, HPC, HD + 1], F16, tag="vp")  # V + ones col
    att = [qkv.tile([128, NQ], F16, tag=f"att{p}", name=f"att{p}") for p in range(PAIRS)]
    nc.vector.memset(vp_sb[:, :, :, HD:HD + 1], 1.0)

    scale = 1.0 / float(np.sqrt(HD))
    ex = {}

    def emit_quad(p, c, tkm):
        """One scoresT quad (2 heads x 128 keys x 512 queries) + fused exp."""
        ps = psum.tile([128, 2, 512], F32, tag="quad", bufs=2,
                       name=f"qps_{p}_{c}_{tkm}")
        et = expool.tile([128, 2, 512], F16, tag="ex", name=f"ex_{p}_{c}_{tkm}")
        for hh in range(2):
            r0 = hh * 64
            for mh in range(2):
                c0 = mh * 64
                nc.tensor.matmul(
                    out=ps[c0:c0 + 64, hh, :],
                    lhsT=kt[p][r0:r0 + 64, tkm * 128 + c0:tkm * 128 + c0 + 64],
                    rhs=qt[p][r0:r0 + 64, c * 512:(c + 1) * 512],
                    start=True, stop=True,
                    tile_position=(r0, c0))
        nc.scalar.activation(out=et[:], in_=ps[:],
                             func=mybir.ActivationFunctionType.Exp,
                             scale=scale, bias=zb[:])
        ex[(p, c, tkm)] = et

    # early quads: all of chunk 0 plus pair 0 of chunk 1, interleaved into the
    # projection chains so the exp stream runs nonstop from ~22us; chunk-0
    # pairs first so each PV iteration's exps complete as early as possible
    early = [(p, 0, k) for p in range(PAIRS) for k in range(TKT)]
    early += [(0, 1, k) for k in range(TKT)]
    early.sort(key=lambda t: (t[1], t[0]))
    ei = 0

    def pop_quads(n, ready_pairs):
        nonlocal ei
        while n > 0 and ei < len(early):
            p, c, k = early[ei]
            if p >= ready_pairs:
                return
            emit_quad(p, c, k)
            ei += 1
            n -= 1

    # ---- QK projections: 2-chain groups (both 512-query chunks of one
    # tensor) sharing LDWEIGHTS, k-outer so the PE tracks the DMA frontier.
    # Group order (0,q),(1,q),(0,k),(1,k),(2,q),(2,k),(3,q),(3,k): pair 1's
    # q-chains (xq + ungated wq1 only) fill pair 0's xk DMA window with real
    # work, so the PE never idles long enough to re-throttle.
    ready = 0
    for m, gi in ((0, 0), (1, 0), (0, 1), (1, 1),
                  (2, 0), (2, 1), (3, 0), (3, 1)):
        x_t, w_sb, dst = ((xq_t, wq_t[m], qt[m]) if gi == 0 else
                          (xk_t, wk_t[m], kt[m]))
        # quads of completed pairs between chain groups keep the exp
        # stream fed while this pair's projections accumulate
        pop_quads(2 if ready else 0, ready_pairs=ready)
        pp = [psum.tile([128, 512], F32, tag="ps", bufs=2,
                        name=f"ps_p{m}_{gi}_{n}") for n in range(NCH)]
        for k in range(KT):
            if k == 4:
                pop_quads(1, ready_pairs=ready)
            for n in range(NCH):
                nc.tensor.matmul(out=pp[n][:],
                                 lhsT=w_sb[:, k, :],
                                 rhs=x_t[k][:, n * 512:(n + 1) * 512],
                                 start=(k == 0), stop=(k == KT - 1))
        for n in range(NCH):
            nc.vector.tensor_copy(out=dst[:, n * 512:(n + 1) * 512],
                                  in_=pp[n][:])
        if gi == 1:
            # pair m's qt/kt both complete: its quads become poppable
            ready = m + 1
            pop_quads(2, ready_pairs=ready)

    # xv DMAs (sync queue): the tiles reuse the xq ring slots, released as
    # pair 3's q-chains finish reading each k-tile
    for k in range(KT):
        xv_t.append(xpool.tile([128, NK], F16, tag="x", name=f"xv{k}"))
        nc.sync.dma_start(out=xv_t[k][:], in_=xvT[k * 128:(k + 1) * 128, :])
    for p in range(PAIRS):
        nc.sync.dma_start(out=wo_sb[:, p, :], in_=wo[p * 128:(p + 1) * 128, :])

    # ---- V projection: 4 groups of 2 token-chains on the 2-slot ps ring ----
    for g in range(4):
        chains = []
        for t in range(g * 2, g * 2 + 2):
            psv = psum.tile([128, 512], F32, tag="ps", bufs=2, name=f"psv_{t}")
            chains.append((psv, t))
        for k in range(KT):
            if k in (3, 6):
                pop_quads(1, ready_pairs=PAIRS)
            for psv, t in chains:
                nc.tensor.matmul(out=psv[:],
                                 lhsT=xv_t[k][:, t * 128:(t + 1) * 128],
                                 rhs=wv_sb[:, k, :], start=(k == 0),
                                 stop=(k == KT - 1))
        for psv, t in chains:
            nc.vector.tensor_copy(
                out=vp_sb[:, t, :, 0:HD],
                in_=psv[:].rearrange("p (h d) -> p h d", h=HPC))
            pop_quads(3, ready_pairs=PAIRS)
    # any leftover early quads
    pop_quads(len(early), ready_pairs=PAIRS)

    def norm_dve(p, c, pvq):
        """DVE half of the softmax normalization: PV copies (releasing the
        pvq banks), denominator row, reciprocal, f16 cast."""
        pvsb = nrm.tile([65, 2, 512], F16, tag="pvsb", name=f"pvsb_{p}_{c}")
        nc.vector.tensor_copy(out=pvsb[:, 0, :], in_=pvq[0:65, 0, :])
        nc.vector.tensor_copy(out=pvsb[:, 1, :], in_=pvq[0:65, 1, :])
        den2 = nrm.tile([1, 1024], F32, tag="den2", name=f"den2_{p}_{c}")
        nc.vector.tensor_copy(
            out=den2[:].rearrange("p (h q) -> p h q", h=2),
            in_=pvsb[64:65, :, :])
        rec2 = nrm.tile([1, 1024], F32, tag="rec2", name=f"rec2_{p}_{c}")
        nc.vector.reciprocal_approx_accurate(out=rec2[:], in_=den2[:],
                                             scratch=rscr2[:])
        rec2h = nrm.tile([1, 1024], F16, tag="rec2h", name=f"rec2h_{p}_{c}")
        nc.vector.tensor_copy(out=rec2h[:], in_=rec2[:])
        return (p, c, pvsb, rec2h)

    def norm_pe(pend):
        """PE half, applied one iteration later so the broadcast matmuls
        never wait on the DVE chain at the head of the PE queue: two
        concurrent col-tiled K=1 broadcasts, then the normalizing muls."""
        p, c, pvsb, rec2h = pend
        cs = slice(c * 512, (c + 1) * 512)
        rb = psum.tile([128, 512], F32, tag="ps", bufs=2, name=f"rb_{p}_{c}")
        nc.tensor.matmul(out=rb[0:64, :], lhsT=selA[:], rhs=rec2h[:, 0:512],
                         start=True, stop=True, tile_position=(0, 0))
        nc.tensor.matmul(out=rb[64:128, :], lhsT=selA[:], rhs=rec2h[:, 512:1024],
                         start=True, stop=True, tile_position=(0, 64))
        nc.vector.tensor_mul(out=att[p][0:64, cs], in0=pvsb[0:64, 0, :],
                             in1=rb[0:64, :])
        nc.vector.tensor_mul(out=att[p][64:128, cs], in0=pvsb[0:64, 1, :],
                             in1=rb[64:128, :])

    tiles_w = [(4 + qi, half) for qi in range(4) for half in range(2)]
    psos_w = []

    def emit_out_wide_pre():
        """Final-chunk out-projection, phase 1: ranks p4=0..2 for 4 output
        tiles in the quad pool — emitted BEFORE the last normalization so
        they aren't queued behind its broadcast matmuls."""
        for j in range(2):
            qa = psum.tile([128, 2, 512], F32, tag="quad", bufs=2,
                           name=f"oq_{j}")
            psos_w.extend([qa[:, 0, :], qa[:, 1, :]])
        for p4 in range(PAIRS - 1):
            for ti in range(4):
                q, half = tiles_w[ti]
                nc.tensor.matmul(out=psos_w[ti],
                                 lhsT=att[p4][:, q * 128:(q + 1) * 128],
                                 rhs=wo_sb[:, p4, half * 512:(half + 1) * 512],
                                 start=(p4 == 0), stop=False)

    def emit_out_wide_post():
        """Phase 2 (after the last norm's allocations): the pv slot + 2
        ps-ring tiles for tiles 4..7, then rank 3 everywhere, then copies."""
        ov = psum.tile([128, 2, 512], F32, tag="pv", bufs=1, name="ov")
        psos_w.extend([ov[:, 0, :], ov[:, 1, :]])
        for j in range(2):
            psos_w.append(psum.tile([128, 512], F32, tag="ps", bufs=2,
                                    name=f"os_{j}"))
        for p4 in range(PAIRS - 1):
            for ti in (4, 5, 6, 7):
                q, half = tiles_w[ti]
                nc.tensor.matmul(out=psos_w[ti],
                                 lhsT=att[p4][:, q * 128:(q + 1) * 128],
                                 rhs=wo_sb[:, p4, half * 512:(half + 1) * 512],
                                 start=(p4 == 0), stop=False)
        for ti in range(8):
            q, half = tiles_w[ti]
            nc.tensor.matmul(out=psos_w[ti],
                             lhsT=att[PAIRS - 1][:, q * 128:(q + 1) * 128],
                             rhs=wo_sb[:, PAIRS - 1, half * 512:(half + 1) * 512],
                             start=False, stop=True)
        for i, ((q, half), pso) in enumerate(zip(tiles_w, psos_w)):
            ot = ost.tile([128, 512], F16, tag="ot", name=f"ot_{q}_{half}")
            # exp stream is drained here: split copies scalar/vector
            if i % 2 == 0:
                nc.scalar.copy(out=ot[:], in_=pso)
            else:
                nc.vector.tensor_copy(out=ot[:], in_=pso)
            eng = nc.sync if i % 2 == 0 else nc.gpsimd
            eng.dma_start(out=out[q * 128:(q + 1) * 128,
                                  half * 512:(half + 1) * 512], in_=ot[:])

    tiles0 = [(qi, half) for qi in range(4) for half in range(2)]

    def emit_out0_part(ti_list):
        """2-3 chunk-0 out-projection tiles on the ps ring; spread across
        iterations 4-6 so their ring stalls hide in the exp-paced slack."""
        for j, ti in enumerate(ti_list):
            q, half = tiles0[ti]
            pso = psum.tile([128, 512], F32, tag="ps", bufs=2,
                            name=f"pso_{q}_{half}")
            for p4 in range(PAIRS):
                nc.tensor.matmul(out=pso[:],
                                 lhsT=att[p4][:, q * 128:(q + 1) * 128],
                                 rhs=wo_sb[:, p4, half * 512:(half + 1) * 512],
                                 start=(p4 == 0), stop=(p4 == PAIRS - 1))
            ot = ost.tile([128, 512], F16, tag="ot", name=f"ot_{q}_{half}")
            nc.vector.tensor_copy(out=ot[:], in_=pso[:])
            eng = nc.sync if ti % 2 == 0 else nc.gpsimd
            eng.dma_start(out=out[q * 128:(q + 1) * 128,
                                  half * 512:(half + 1) * 512], in_=ot[:])

    # ---- pipelined (pair, chunk) iterations ----
    # PV pairs write a dedicated 2-bank PSUM tile. Emission order per
    # iteration: PV, the PREVIOUS iteration's norm-PE half (its DVE inputs
    # are ready by now, so the broadcast matmuls never stall the PE queue),
    # quads for seq[i+5], out-proj blocks, then this iteration's norm-DVE.
    seq = [(p, c) for c in range(NCH) for p in range(PAIRS)]
    pending = None
    for i, (p, c) in enumerate(seq):
        pvq = psum.tile([128, 2, 512], F32, tag="pv", bufs=1,
                        name=f"pv_{p}_{c}")
        for k in range(TKT):
            et = ex[(p, c, k)]
            for hh in range(2):
                h = p * 2 + hh
                nc.tensor.matmul(out=pvq[0:65, hh, :],
                                 lhsT=vp_sb[:, k, h, :],
                                 rhs=et[:, hh, :],
                                 start=(k == 0), stop=(k == TKT - 1))
            del ex[(p, c, k)]
        if pending is not None:
            norm_pe(pending)
        if i + 5 < len(seq):
            for k in range(TKT):
                emit_quad(seq[i + 5][0], seq[i + 5][1], k)
        if i == 4:
            emit_out0_part([0, 1, 2])
        elif i == 5:
            emit_out0_part([3, 4, 5])
        elif i == 6:
            emit_out0_part([6, 7])
        if i == len(seq) - 1:
            emit_out_wide_pre()
        pending = norm_dve(p, c, pvq)
    norm_pe(pending)
    emit_out_wide_post()
    ctx.close()


_NC_CACHE = None


def build():
    global _NC_CACHE
    if _NC_CACHE is None:
        nc = bacc.Bacc("TRN2", target_bir_lowering=False, debug=False,
                       num_devices=NCORES)
        with tile.TileContext(nc) as tc:
            _emit(tc)
        nc.compile()
        _NC_CACHE = nc
    return _NC_CACHE


def make_in_maps(inputs):
    q = np.asarray(inputs["query_tokens"], dtype=np.float32)
    kk = np.asarray(inputs["key_tokens"], dtype=np.float32)
    v = np.asarray(inputs["value_tokens"], dtype=np.float32)
    Wq = np.asarray(inputs["Wq"], dtype=np.float32)
    Wk = np.asarray(inputs["Wk"], dtype=np.float32)
    Wv = np.asarray(inputs["Wv"], dtype=np.float32)
    Wo = np.asarray(inputs["Wo"], dtype=np.float32)

    def swizzle(w_cols):
        # [1024, 512] -> [pair m][sbuf partition p][k-tile k][d]: block m is a
        # contiguous [128, 1024] so one DMA per pair lands pair-major
        a = w_cols.reshape(8, 128, 4, 128).transpose(2, 1, 0, 3)
        return np.ascontiguousarray(a.reshape(512, 1024)).astype(np.float16)

    qT = [np.ascontiguousarray(q[b].T).astype(np.float16) for b in range(B)]
    kT = [np.ascontiguousarray(kk[b].T).astype(np.float16) for b in range(B)]
    vT = [np.ascontiguousarray(v[b].T).astype(np.float16) for b in range(B)]
    wq_g = [swizzle(Wq[:, g * F:(g + 1) * F]) for g in range(2)]
    wk_g = [swizzle(Wk[:, g * F:(g + 1) * F]) for g in range(2)]
    wv_g = [np.ascontiguousarray(Wv[:, g * F:(g + 1) * F]).astype(np.float16)
            for g in range(2)]
    wo_g = [np.ascontiguousarray(Wo[g * F:(g + 1) * F, :]).astype(np.float16)
            for g in range(2)]

    in_maps = []
    for c in range(NCORES):
        b, g = c // 2, c % 2
        in_maps.append({
            "xqT": qT[b], "xkT": kT[b], "xvT": vT[b],
            "wq": wq_g[g], "wk": wk_g[g], "wv": wv_g[g], "wo": wo_g[g],
        })
    return in_maps


def combine(results, bo):
    out = np.zeros((B, NQ, D), dtype=np.float32)
    for c in range(NCORES):
        out[c // 2] += results[c]["out"].astype(np.float32)
    out += np.asarray(bo, dtype=np.float32)[None, None, :]
    return out


def kernel(**inputs):
    nc = build()
    in_maps = make_in_maps(inputs)
    res = run_bass_kernel_spmd(nc, in_maps, list(range(NCORES)))
    return combine(res.results, inputs["bo"])


# revision 35
# speedup vs baseline: 1.1746x; 1.0142x over previous
"""Trainium2 Bass kernel for nn_CrossAttention (B=4, NQ=NK=1024, D=1024, H=16).

Sharding: 8 cores = 4 batches x 2 head-groups (8 heads each). Per core:
  - inputs arrive pre-transposed/sliced on host (free): xqT/xkT/xvT [D, T] fp16,
    Wq/Wk/Wv column slices [D, 512] fp16, Wo row slice [512, D] fp16.
  - warm-up matmuls on a zeroed scratch tile run from t~7us (during the DMA
    fill) so the PE HAM clock-gate is released before real matmuls start.
  - input DMAs spread over 4 hw queues (gpsimd/sync/vector/scalar) with pair-0
    weights at the head of the gpsimd queue and xq/xk each split even/odd
    across two queues, so the projection k-frontier is never DMA-starved.
  - projections produce Q^T/K^T per head-pair [128, T] (lhsT = W slice, rhs =
    xT) in 2-chain groups (shared LDWEIGHTS) on a 2-slot PSUM ring.
  - scores computed transposed (scoresT [Tk, Tq]) as 4-way tile_position-packed
    quads (2 heads x 2 M-halves, K=64 each) into 2-bank PSUM tiles (3-deep
    ring); ONE fused exp activation per quad (free size 1024).
  - the exp stream starts at ~21us and must never starve: 40 early quads
    (all of chunk 0 + pair 0 of chunk 1) are interleaved into the projection
    chains, the rest follow with +5 lookahead in the PV loop.
  - denominators via an augmented ones-column in V (row 64 of the PV output);
    reciprocal via 2-op approx on [2,512]; the per-query reciprocals are
    broadcast to 128 partitions with ONE K=2 PE matmul (selector lhsT), so
    gpsimd is off the critical path entirely.
  - per query-chunk out-projection overlaps the next chunk's scores/PV; final
    chunk accumulates p4-major across 8 PSUM banks, copies alternate
    scalar/vector, output DMAs alternate sync/gpsimd.
  - host sums the two head-group partials per batch and adds the bias.
All matmuls fp16 (1 cycle/row on PE), accumulation fp32 in PSUM.
"""
import sys

sys.path.insert(0, "/opt/trn_rl_repo")

from contextlib import ExitStack

import numpy as np

import concourse.bass as bass
import concourse.tile as tile
from concourse import bacc, mybir
from concourse.bass_utils import run_bass_kernel_spmd

F32 = mybir.dt.float32
F16 = mybir.dt.float16

B, NQ, NK, D, H, HD = 4, 1024, 1024, 1024, 16, 64
NCORES = 8
HPC = 8          # heads per core
F = HPC * HD     # 512: per-core projection width
KT = D // 128    # 8 k-tiles over D
PAIRS = HPC // 2  # 4 head pairs
TKT = NK // 128  # 8 tiles over key tokens
NCH = NQ // 512  # 2 query chunks

N_WARM = 9       # warm-up matmuls (~3.8us cold) to release the HAM throttle
H1_DIRECT = True  # DVE mul writing partitions 64-127 from inputs at 0-63


def _emit(tc):
    nc = tc.nc
    ctx = ExitStack()

    xqT = nc.dram_tensor("xqT", [D, NQ], F16, kind="ExternalInput").ap()
    xkT = nc.dram_tensor("xkT", [D, NK], F16, kind="ExternalInput").ap()
    xvT = nc.dram_tensor("xvT", [D, NK], F16, kind="ExternalInput").ap()
    # wq/wk host-swizzled to [m-pair][sbuf-partition][k-tile*128]: one
    # contiguous 256KB DMA per pair
    wq = nc.dram_tensor("wq", [PAIRS * 128, D], F16, kind="ExternalInput").ap()
    wk = nc.dram_tensor("wk", [PAIRS * 128, D], F16, kind="ExternalInput").ap()
    wv = nc.dram_tensor("wv", [D, F], F16, kind="ExternalInput").ap()
    wo = nc.dram_tensor("wo", [F, D], F16, kind="ExternalInput").ap()
    out = nc.dram_tensor("out", [NQ, D], F16, kind="ExternalOutput").ap()

    wpool = ctx.enter_context(tc.tile_pool(name="wpool", bufs=1))
    qkv = ctx.enter_context(tc.tile_pool(name="qkv", bufs=1))
    xpool = ctx.enter_context(tc.tile_pool(name="xpool", bufs=16))
    expool = ctx.enter_context(tc.tile_pool(name="expool", bufs=42))
    psum = ctx.enter_context(tc.tile_pool(name="psum", bufs=1, space="PSUM"))
    nrm = ctx.enter_context(tc.tile_pool(name="nrm", bufs=2))
    ost = ctx.enter_context(tc.tile_pool(name="ost", bufs=4))

    # zero-bias AP for activations (avoids the const-page TENSOR_LOAD)
    zb = nrm.tile([128, 1], F32, tag="zb", bufs=1)
    nc.vector.memset(zb[:], 0.0)
    # ones selector for the K=1 reciprocal-broadcast matmuls
    selA = nrm.tile([1, 64], F16, tag="selA", bufs=1)
    nc.vector.memset(selA[:], 1.0)
    # warm-up scratch (memset on gpsimd: ready before the vector preamble)
    warm = nrm.tile([128, 512], F16, tag="warm", bufs=1)
    nc.gpsimd.memset(warm[:], 0.0)
    rscr2 = nrm.tile([1, 1024], F32, tag="rscr2", bufs=1)
    # preload the exp table set during the DMA fill (saves ~2.7us at the
    # first real exp)
    tload = nrm.tile([128, 1], F16, tag="tload", bufs=1)
    nc.scalar.activation(out=tload[:], in_=zb[:],
                         func=mybir.ActivationFunctionType.Exp,
                         scale=1.0, bias=zb[:])

    # ---- warm-up matmuls: PE busy from t~7.3us so HAM hits K=8/8 by the time
    # the first real projection matmul runs
    for i in range(N_WARM):
        wps = psum.tile([128, 512], F32, tag="ps", bufs=2, name=f"warm{i}")
        nc.tensor.matmul(out=wps[:], lhsT=warm[:, 0:128], rhs=warm[:],
                         start=True, stop=True)

    # ---- input DMAs over 3 queues. x tiles get the gpsimd AND sync queues
    # (even/odd split per tensor); weights ride the scalar queue with pairs
    # 2/3 in a 2-slot ring so their transfers self-gate (WAR semaphore)
    # behind pair 0/1's projection reads instead of crowding the x stream.
    wq_t = [wpool.tile([128, KT, 128], F16, tag="wqr", bufs=2, name=f"wqr{m}")
            for m in range(PAIRS)]
    wk_t = [wpool.tile([128, KT, 128], F16, tag="wkr", bufs=2, name=f"wkr{m}")
            for m in range(PAIRS)]
    wv_sb = wpool.tile([128, KT, F], F16, tag="wv")
    wo_sb = wpool.tile([128, PAIRS, D], F16, tag="wo")

    xq_t, xk_t, xv_t = [], [], []
    for k in range(KT):
        xq_t.append(xpool.tile([128, NQ], F16, tag="x", name=f"xq{k}"))
        xk_t.append(xpool.tile([128, NK], F16, tag="x", name=f"xk{k}"))

    # The gpsimd software-DGE queue outcompetes the HW-DGE queues on the
    # shared DMA engines, so the ENTIRE phase-1 critical sequence rides it
    # in exact consumption order; wv (needed ~20us later) goes to sync.
    nc.gpsimd.dma_start(out=wq_t[0][:], in_=wq[0:128, :])
    nc.gpsimd.dma_start(out=wq_t[1][:], in_=wq[128:256, :])
    for k in range(KT):
        nc.gpsimd.dma_start(out=xq_t[k][:], in_=xqT[k * 128:(k + 1) * 128, :])
    nc.gpsimd.dma_start(out=wk_t[0][:], in_=wk[0:128, :])
    nc.gpsimd.dma_start(out=wk_t[1][:], in_=wk[128:256, :])
    for k in range(KT):
        nc.gpsimd.dma_start(out=xk_t[k][:], in_=xkT[k * 128:(k + 1) * 128, :])
    # sync queue: wv (starved while the gpsimd queue is active, flows right
    # after; needed only by the V projection)
    for k in range(KT):
        nc.sync.dma_start(out=wv_sb[:, k, :], in_=wv[k * 128:(k + 1) * 128, :])
    # scalar queue: only the ring-gated pair 2/3 weights — their descriptors
    # self-gate (WAR) behind pair 0/1's reads, so this queue never competes
    # with the phase-1 x stream
    for m in range(2, PAIRS):
        nc.scalar.dma_start(out=wq_t[m][:], in_=wq[m * 128:(m + 1) * 128, :])
        nc.scalar.dma_start(out=wk_t[m][:], in_=wk[m * 128:(m + 1) * 128, :])

    # ---- persistent intermediates ----
    qt = [qkv.tile([128, NQ], F16, tag=f"qt{p}", name=f"qt{p}") for p in range(PAIRS)]
    kt = [qkv.tile([128, NK], F16, tag=f"kt{p}", name=f"kt{p}") for p in range(PAIRS)]
    vp_sb = qkv.tile([128, TKT, HPC, HD + 1], F16, tag="vp")  # V + ones col
    att = [qkv.tile([128, NQ], F16, tag=f"att{p}", name=f"att{p}") for p in range(PAIRS)]
    nc.vector.memset(vp_sb[:, :, :, HD:HD + 1], 1.0)

    scale = 1.0 / float(np.sqrt(HD))
    ex = {}

    def emit_quad(p, c, tkm):
        """One scoresT quad (2 heads x 128 keys x 512 queries) + fused exp."""
        ps = psum.tile([128, 2, 512], F32, tag="quad", bufs=2,
                       name=f"qps_{p}_{c}_{tkm}")
        et = expool.tile([128, 2, 512], F16, tag="ex", name=f"ex_{p}_{c}_{tkm}")
        for hh in range(2):
            r0 = hh * 64
            for mh in range(2):
                c0 = mh * 64
                nc.tensor.matmul(
                    out=ps[c0:c0 + 64, hh, :],
                    lhsT=kt[p][r0:r0 + 64, tkm * 128 + c0:tkm * 128 + c0 + 64],
                    rhs=qt[p][r0:r0 + 64, c * 512:(c + 1) * 512],
                    start=True, stop=True,
                    tile_position=(r0, c0))
        nc.scalar.activation(out=et[:], in_=ps[:],
                             func=mybir.ActivationFunctionType.Exp,
                             scale=scale, bias=zb[:])
        ex[(p, c, tkm)] = et

    # early quads: all of chunk 0 plus pair 0 of chunk 1, interleaved into the
    # projection chains so the exp stream runs nonstop from ~22us; chunk-0
    # pairs first so each PV iteration's exps complete as early as possible
    early = [(p, 0, k) for p in range(PAIRS) for k in range(TKT)]
    early += [(0, 1, k) for k in range(TKT)]
    early.sort(key=lambda t: (t[1], t[0]))
    ei = 0

    def pop_quads(n, ready_pairs):
        nonlocal ei
        while n > 0 and ei < len(early):
            p, c, k = early[ei]
            if p >= ready_pairs:
                return
            emit_quad(p, c, k)
            ei += 1
            n -= 1

    # ---- QK projections: 2-chain groups (both 512-query chunks of one
    # tensor) sharing LDWEIGHTS, k-outer so the PE tracks the DMA frontier.
    # Group order (0,q),(1,q),(0,k),(1,k),(2,q),(2,k),(3,q),(3,k): pair 1's
    # q-chains (xq + ungated wq1 only) fill pair 0's xk DMA window with real
    # work, so the PE never idles long enough to re-throttle.
    ready = 0
    for m, gi in ((0, 0), (1, 0), (0, 1), (1, 1),
                  (2, 0), (2, 1), (3, 0), (3, 1)):
        x_t, w_sb, dst = ((xq_t, wq_t[m], qt[m]) if gi == 0 else
                          (xk_t, wk_t[m], kt[m]))
        # quads of completed pairs between chain groups keep the exp
        # stream fed while this pair's projections accumulate
        pop_quads(2 if ready else 0, ready_pairs=ready)
        pp = [psum.tile([128, 512], F32, tag="ps", bufs=2,
                        name=f"ps_p{m}_{gi}_{n}") for n in range(NCH)]
        for k in range(KT):
            if k == 4:
                pop_quads(1, ready_pairs=ready)
            for n in range(NCH):
                nc.tensor.matmul(out=pp[n][:],
                                 lhsT=w_sb[:, k, :],
                                 rhs=x_t[k][:, n * 512:(n + 1) * 512],
                                 start=(k == 0), stop=(k == KT - 1))
        for n in range(NCH):
            nc.vector.tensor_copy(out=dst[:, n * 512:(n + 1) * 512],
                                  in_=pp[n][:])
        if gi == 1:
            # pair m's qt/kt both complete: its quads become poppable
            ready = m + 1
            pop_quads(2, ready_pairs=ready)

    # xv DMAs (sync queue): the tiles reuse the xq ring slots, released as
    # pair 3's q-chains finish reading each k-tile
    for k in range(KT):
        xv_t.append(xpool.tile([128, NK], F16, tag="x", name=f"xv{k}"))
        nc.sync.dma_start(out=xv_t[k][:], in_=xvT[k * 128:(k + 1) * 128, :])
    for p in range(PAIRS):
        nc.sync.dma_start(out=wo_sb[:, p, :], in_=wo[p * 128:(p + 1) * 128, :])

    # ---- V projection: 4 groups of 2 token-chains on the 2-slot ps ring ----
    for g in range(4):
        chains = []
        for t in range(g * 2, g * 2 + 2):
            psv = psum.tile([128, 512], F32, tag="ps", bufs=2, name=f"psv_{t}")
            chains.append((psv, t))
        for k in range(KT):
            if k in (3, 6):
                pop_quads(1, ready_pairs=PAIRS)
            for psv, t in chains:
                nc.tensor.matmul(out=psv[:],
                                 lhsT=xv_t[k][:, t * 128:(t + 1) * 128],
                                 rhs=wv_sb[:, k, :], start=(k == 0),
                                 stop=(k == KT - 1))
        for psv, t in chains:
            nc.vector.tensor_copy(
                out=vp_sb[:, t, :, 0:HD],
                in_=psv[:].rearrange("p (h d) -> p h d", h=HPC))
            pop_quads(3, ready_pairs=PAIRS)
    # any leftover early quads
    pop_quads(len(early), ready_pairs=PAIRS)

    def norm_dve(p, c, pvq):
        """DVE half of the softmax normalization: PV copies (releasing the
        pvq banks), denominator row, reciprocal, f16 cast."""
        pvsb = nrm.tile([65, 2, 512], F16, tag="pvsb", name=f"pvsb_{p}_{c}")
        nc.vector.tensor_copy(out=pvsb[:, 0, :], in_=pvq[0:65, 0, :])
        nc.vector.tensor_copy(out=pvsb[:, 1, :], in_=pvq[0:65, 1, :])
        den2 = nrm.tile([1, 1024], F32, tag="den2", name=f"den2_{p}_{c}")
        nc.vector.tensor_copy(
            out=den2[:].rearrange("p (h q) -> p h q", h=2),
            in_=pvsb[64:65, :, :])
        rec2 = nrm.tile([1, 1024], F32, tag="rec2", name=f"rec2_{p}_{c}")
        nc.vector.reciprocal_approx_accurate(out=rec2[:], in_=den2[:],
                                             scratch=rscr2[:])
        rec2h = nrm.tile([1, 1024], F16, tag="rec2h", name=f"rec2h_{p}_{c}")
        nc.vector.tensor_copy(out=rec2h[:], in_=rec2[:])
        return (p, c, pvsb, rec2h)

    def norm_pe(pend):
        """PE half, applied one iteration later so the broadcast matmuls
        never wait on the DVE chain at the head of the PE queue: two
        concurrent col-tiled K=1 broadcasts, then the normalizing muls."""
        p, c, pvsb, rec2h = pend
        cs = slice(c * 512, (c + 1) * 512)
        rb = psum.tile([128, 512], F32, tag="ps", bufs=2, name=f"rb_{p}_{c}")
        nc.tensor.matmul(out=rb[0:64, :], lhsT=selA[:], rhs=rec2h[:, 0:512],
                         start=True, stop=True, tile_position=(0, 0))
        nc.tensor.matmul(out=rb[64:128, :], lhsT=selA[:], rhs=rec2h[:, 512:1024],
                         start=True, stop=True, tile_position=(0, 64))
        nc.vector.tensor_mul(out=att[p][0:64, cs], in0=pvsb[0:64, 0, :],
                             in1=rb[0:64, :])
        nc.vector.tensor_mul(out=att[p][64:128, cs], in0=pvsb[0:64, 1, :],
                             in1=rb[64:128, :])

    tiles_w = [(4 + qi, half) for qi in range(4) for half in range(2)]
    psos_w = []

    def emit_out_wide_pre():
        """Final-chunk out-projection, phase 1: ranks p4=0..2 for 4 output
        tiles in the quad pool — emitted BEFORE the last normalization so
        they aren't queued behind its broadcast matmuls."""
        for j in range(2):
            qa = psum.tile([128, 2, 512], F32, tag="quad", bufs=2,
                           name=f"oq_{j}")
            psos_w.extend([qa[:, 0, :], qa[:, 1, :]])
        for p4 in range(PAIRS - 1):
            for ti in range(4):
                q, half = tiles_w[ti]
                nc.tensor.matmul(out=psos_w[ti],
                                 lhsT=att[p4][:, q * 128:(q + 1) * 128],
                                 rhs=wo_sb[:, p4, half * 512:(half + 1) * 512],
                                 start=(p4 == 0), stop=False)

    def emit_out_wide_post():
        """Phase 2 (after the last norm's allocations): the pv slot + 2
        ps-ring tiles for tiles 4..7, then rank 3 everywhere, then copies."""
        ov = psum.tile([128, 2, 512], F32, tag="pv", bufs=1, name="ov")
        psos_w.extend([ov[:, 0, :], ov[:, 1, :]])
        for j in range(2):
            psos_w.append(psum.tile([128, 512], F32, tag="ps", bufs=2,
                                    name=f"os_{j}"))
        for p4 in range(PAIRS - 1):
            for ti in (4, 5, 6, 7):
                q, half = tiles_w[ti]
                nc.tensor.matmul(out=psos_w[ti],
                                 lhsT=att[p4][:, q * 128:(q + 1) * 128],
                                 rhs=wo_sb[:, p4, half * 512:(half + 1) * 512],
                                 start=(p4 == 0), stop=False)
        for ti in range(8):
            q, half = tiles_w[ti]
            nc.tensor.matmul(out=psos_w[ti],
                             lhsT=att[PAIRS - 1][:, q * 128:(q + 1) * 128],
                             rhs=wo_sb[:, PAIRS - 1, half * 512:(half + 1) * 512],
                             start=False, stop=True)
        for i, ((q, half), pso) in enumerate(zip(tiles_w, psos_w)):
            ot = ost.tile([128, 512], F16, tag="ot", name=f"ot_{q}_{half}")
            # exp stream is drained here: split copies scalar/vector
            if i % 2 == 0:
                nc.scalar.copy(out=ot[:], in_=pso)
            else:
                nc.vector.tensor_copy(out=ot[:], in_=pso)
            eng = nc.sync if i % 2 == 0 else nc.gpsimd
            eng.dma_start(out=out[q * 128:(q + 1) * 128,
                                  half * 512:(half + 1) * 512], in_=ot[:])

    tiles0 = [(qi, half) for qi in range(4) for half in range(2)]

    def emit_out0_part(ti_list):
        """2-3 chunk-0 out-projection tiles on the ps ring; spread across
        iterations 4-6 so their ring stalls hide in the exp-paced slack."""
        for j, ti in enumerate(ti_list):
            q, half = tiles0[ti]
            pso = psum.tile([128, 512], F32, tag="ps", bufs=2,
                            name=f"pso_{q}_{half}")
            for p4 in range(PAIRS):
                nc.tensor.matmul(out=pso[:],
                                 lhsT=att[p4][:, q * 128:(q + 1) * 128],
                                 rhs=wo_sb[:, p4, half * 512:(half + 1) * 512],
                                 start=(p4 == 0), stop=(p4 == PAIRS - 1))
            ot = ost.tile([128, 512], F16, tag="ot", name=f"ot_{q}_{half}")
            nc.vector.tensor_copy(out=ot[:], in_=pso[:])
            eng = nc.sync if ti % 2 == 0 else nc.gpsimd
            eng.dma_start(out=out[q * 128:(q + 1) * 128,
                                  half * 512:(half + 1) * 512], in_=ot[:])

    # ---- pipelined (pair, chunk) iterations ----
    # PV pairs write a dedicated 2-bank PSUM tile. Emission order per
    # iteration: PV, the PREVIOUS iteration's norm-PE half (its DVE inputs
    # are ready by now, so the broadcast matmuls never stall the PE queue),
    # quads for seq[i+5], out-proj blocks, then this iteration's norm-DVE.
    seq = [(p, c) for c in range(NCH) for p in range(PAIRS)]
    pending = None
    for i, (p, c) in enumerate(seq):
        pvq = psum.tile([128, 2, 512], F32, tag="pv", bufs=1,
                        name=f"pv_{p}_{c}")
        for k in range(TKT):
            et = ex[(p, c, k)]
            for hh in range(2):
                h = p * 2 + hh
                nc.tensor.matmul(out=pvq[0:65, hh, :],
                                 lhsT=vp_sb[:, k, h, :],
                                 rhs=et[:, hh, :],
                                 start=(k == 0), stop=(k == TKT - 1))
            del ex[(p, c, k)]
        if pending is not None:
            norm_pe(pending)
        if i + 5 < len(seq):
            for k in range(TKT):
                emit_quad(seq[i + 5][0], seq[i + 5][1], k)
        if i == 4:
            emit_out0_part([0, 1, 2])
        elif i == 5:
            emit_out0_part([3, 4, 5])
        elif i == 6:
            emit_out0_part([6, 7])
        if i == len(seq) - 1:
            emit_out_wide_pre()
        pending = norm_dve(p, c, pvq)
    norm_pe(pending)
    emit_out_wide_post()
    ctx.close()


_NC_CACHE = None


def build():
    global _NC_CACHE
    if _NC_CACHE is None:
        nc = bacc.Bacc("TRN2", target_bir_lowering=False, debug=False,
                       num_devices=NCORES)
        with tile.TileContext(nc) as tc:
            _emit(tc)
        nc.compile()
        _NC_CACHE = nc
    return _NC_CACHE


def make_in_maps(inputs):
    q = np.asarray(inputs["query_tokens"], dtype=np.float32)
    kk = np.asarray(inputs["key_tokens"], dtype=np.float32)
    v = np.asarray(inputs["value_tokens"], dtype=np.float32)
    Wq = np.asarray(inputs["Wq"], dtype=np.float32)
    Wk = np.asarray(inputs["Wk"], dtype=np.float32)
    Wv = np.asarray(inputs["Wv"], dtype=np.float32)
    Wo = np.asarray(inputs["Wo"], dtype=np.float32)

    def swizzle(w_cols):
        # [1024, 512] -> [pair m][sbuf partition p][k-tile k][d]: block m is a
        # contiguous [128, 1024] so one DMA per pair lands pair-major
        a = w_cols.reshape(8, 128, 4, 128).transpose(2, 1, 0, 3)
        return np.ascontiguousarray(a.reshape(512, 1024)).astype(np.float16)

    qT = [np.ascontiguousarray(q[b].T).astype(np.float16) for b in range(B)]
    kT = [np.ascontiguousarray(kk[b].T).astype(np.float16) for b in range(B)]
    vT = [np.ascontiguousarray(v[b].T).astype(np.float16) for b in range(B)]
    wq_g = [swizzle(Wq[:, g * F:(g + 1) * F]) for g in range(2)]
    wk_g = [swizzle(Wk[:, g * F:(g + 1) * F]) for g in range(2)]
    wv_g = [np.ascontiguousarray(Wv[:, g * F:(g + 1) * F]).astype(np.float16)
            for g in range(2)]
    wo_g = [np.ascontiguousarray(Wo[g * F:(g + 1) * F, :]).astype(np.float16)
            for g in range(2)]

    in_maps = []
    for c in range(NCORES):
        b, g = c // 2, c % 2
        in_maps.append({
            "xqT": qT[b], "xkT": kT[b], "xvT": vT[b],
            "wq": wq_g[g], "wk": wk_g[g], "wv": wv_g[g], "wo": wo_g[g],
        })
    return in_maps


def combine(results, bo):
    out = np.zeros((B, NQ, D), dtype=np.float32)
    for c in range(NCORES):
        out[c // 2] += results[c]["out"].astype(np.float32)
    out += np.asarray(bo, dtype=np.float32)[None, None, :]
    return out


def kernel(**inputs):
    nc = build()
    in_maps = make_in_maps(inputs)
    res = run_bass_kernel_spmd(nc, in_maps, list(range(NCORES)))
    return combine(res.results, inputs["bo"])


# revision 39
# speedup vs baseline: 1.1943x; 1.0167x over previous
"""Trainium2 Bass kernel for nn_CrossAttention (B=4, NQ=NK=1024, D=1024, H=16).

Sharding: 8 cores = 4 batches x 2 head-groups (8 heads each). Per core:
  - inputs arrive pre-transposed/sliced on host (free): xqT/xkT/xvT [D, T] fp16,
    Wq/Wk/Wv column slices [D, 512] fp16, Wo row slice [512, D] fp16.
  - warm-up matmuls on a zeroed scratch tile run from t~7us (during the DMA
    fill) so the PE HAM clock-gate is released before real matmuls start.
  - input DMAs spread over 4 hw queues (gpsimd/sync/vector/scalar) with pair-0
    weights at the head of the gpsimd queue and xq/xk each split even/odd
    across two queues, so the projection k-frontier is never DMA-starved.
  - projections produce Q^T/K^T per head-pair [128, T] (lhsT = W slice, rhs =
    xT) in 2-chain groups (shared LDWEIGHTS) on a 2-slot PSUM ring.
  - scores computed transposed (scoresT [Tk, Tq]) as 4-way tile_position-packed
    quads (2 heads x 2 M-halves, K=64 each) into 2-bank PSUM tiles (3-deep
    ring); ONE fused exp activation per quad (free size 1024).
  - the exp stream starts at ~21us and must never starve: 40 early quads
    (all of chunk 0 + pair 0 of chunk 1) are interleaved into the projection
    chains, the rest follow with +5 lookahead in the PV loop.
  - denominators via an augmented ones-column in V (row 64 of the PV output);
    reciprocal via 2-op approx on [2,512]; the per-query reciprocals are
    broadcast to 128 partitions with ONE K=2 PE matmul (selector lhsT), so
    gpsimd is off the critical path entirely.
  - per query-chunk out-projection overlaps the next chunk's scores/PV; final
    chunk accumulates p4-major across 8 PSUM banks, copies alternate
    scalar/vector, output DMAs alternate sync/gpsimd.
  - host sums the two head-group partials per batch and adds the bias.
All matmuls fp16 (1 cycle/row on PE), accumulation fp32 in PSUM.
"""
import sys

sys.path.insert(0, "/opt/trn_rl_repo")

from contextlib import ExitStack

import numpy as np

import concourse.bass as bass
import concourse.tile as tile
from concourse import bacc, mybir
from concourse.bass_utils import run_bass_kernel_spmd

F32 = mybir.dt.float32
F16 = mybir.dt.float16

B, NQ, NK, D, H, HD = 4, 1024, 1024, 1024, 16, 64
NCORES = 8
HPC = 8          # heads per core
F = HPC * HD     # 512: per-core projection width
KT = D // 128    # 8 k-tiles over D
PAIRS = HPC // 2  # 4 head pairs
TKT = NK // 128  # 8 tiles over key tokens
NCH = NQ // 512  # 2 query chunks

N_WARM = 5       # warm-up matmuls to release the HAM throttle
H1_DIRECT = True  # DVE mul writing partitions 64-127 from inputs at 0-63


def _emit(tc):
    nc = tc.nc
    ctx = ExitStack()

    xqT = nc.dram_tensor("xqT", [D, NQ], F16, kind="ExternalInput").ap()
    xkT = nc.dram_tensor("xkT", [D, NK], F16, kind="ExternalInput").ap()
    xvT = nc.dram_tensor("xvT", [D, NK], F16, kind="ExternalInput").ap()
    # wq/wk host-swizzled to [m-pair][sbuf-partition][k-tile*128]: one
    # contiguous 256KB DMA per pair
    wq = nc.dram_tensor("wq", [PAIRS * 128, D], F16, kind="ExternalInput").ap()
    wk = nc.dram_tensor("wk", [PAIRS * 128, D], F16, kind="ExternalInput").ap()
    wv = nc.dram_tensor("wv", [D, F], F16, kind="ExternalInput").ap()
    wo = nc.dram_tensor("wo", [F, D], F16, kind="ExternalInput").ap()
    out = nc.dram_tensor("out", [NQ, D], F16, kind="ExternalOutput").ap()

    wpool = ctx.enter_context(tc.tile_pool(name="wpool", bufs=1))
    qkv = ctx.enter_context(tc.tile_pool(name="qkv", bufs=1))
    xpool = ctx.enter_context(tc.tile_pool(name="xpool", bufs=16))
    expool = ctx.enter_context(tc.tile_pool(name="expool", bufs=42))
    psum = ctx.enter_context(tc.tile_pool(name="psum", bufs=1, space="PSUM"))
    nrm = ctx.enter_context(tc.tile_pool(name="nrm", bufs=2))
    ost = ctx.enter_context(tc.tile_pool(name="ost", bufs=4))

    # zero-bias AP for activations (avoids the const-page TENSOR_LOAD)
    zb = nrm.tile([128, 1], F32, tag="zb", bufs=1)
    nc.vector.memset(zb[:], 0.0)
    # ones selector for the K=1 reciprocal-broadcast matmuls
    selA = nrm.tile([1, 64], F16, tag="selA", bufs=1)
    nc.vector.memset(selA[:], 1.0)
    # warm-up scratch (memset on gpsimd: ready before the vector preamble)
    warm = nrm.tile([128, 512], F16, tag="warm", bufs=1)
    nc.gpsimd.memset(warm[:], 0.0)
    rscr2 = nrm.tile([1, 1024], F32, tag="rscr2", bufs=1)
    # preload the exp table set during the DMA fill (saves ~2.7us at the
    # first real exp)
    tload = nrm.tile([128, 1], F16, tag="tload", bufs=1)
    nc.scalar.activation(out=tload[:], in_=zb[:],
                         func=mybir.ActivationFunctionType.Exp,
                         scale=1.0, bias=zb[:])

    # ---- warm-up matmuls: PE busy from t~7.3us so HAM hits K=8/8 by the time
    # the first real projection matmul runs
    for i in range(N_WARM):
        wps = psum.tile([128, 512], F32, tag="ps", bufs=2, name=f"warm{i}")
        nc.tensor.matmul(out=wps[:], lhsT=warm[:, 0:128], rhs=warm[:],
                         start=True, stop=True)

    # ---- input DMAs over 3 queues. x tiles get the gpsimd AND sync queues
    # (even/odd split per tensor); weights ride the scalar queue with pairs
    # 2/3 in a 2-slot ring so their transfers self-gate (WAR semaphore)
    # behind pair 0/1's projection reads instead of crowding the x stream.
    wq_t = [wpool.tile([128, KT, 128], F16, tag="wqr", bufs=2, name=f"wqr{m}")
            for m in range(PAIRS)]
    wk_t = [wpool.tile([128, KT, 128], F16, tag="wkr", bufs=2, name=f"wkr{m}")
            for m in range(PAIRS)]
    wv_sb = wpool.tile([128, KT, F], F16, tag="wv")
    wo_sb = wpool.tile([128, PAIRS, D], F16, tag="wo")

    xq_t, xk_t, xv_t = [], [], []
    for k in range(KT):
        xq_t.append(xpool.tile([128, NQ], F16, tag="x", name=f"xq{k}"))
        xk_t.append(xpool.tile([128, NK], F16, tag="x", name=f"xk{k}"))

    # The gpsimd software-DGE queue outcompetes the HW-DGE queues on the
    # shared DMA engines, so the ENTIRE phase-1 critical sequence rides it
    # in exact consumption order; wv (needed ~20us later) goes to sync.
    nc.gpsimd.dma_start(out=wq_t[0][:], in_=wq[0:128, :])
    nc.gpsimd.dma_start(out=wq_t[1][:], in_=wq[128:256, :])
    for k in range(KT):
        nc.gpsimd.dma_start(out=xq_t[k][:], in_=xqT[k * 128:(k + 1) * 128, :])
    nc.gpsimd.dma_start(out=wk_t[0][:], in_=wk[0:128, :])
    nc.gpsimd.dma_start(out=wk_t[1][:], in_=wk[128:256, :])
    for k in range(KT):
        nc.gpsimd.dma_start(out=xk_t[k][:], in_=xkT[k * 128:(k + 1) * 128, :])
    # scalar queue: the ring-gated pair 2/3 weights (their descriptors
    # self-gate via WAR behind pair 0/1's reads), then wv — so this queue
    # never competes with the phase-1 x stream
    for m in range(2, PAIRS):
        nc.scalar.dma_start(out=wq_t[m][:], in_=wq[m * 128:(m + 1) * 128, :])
        nc.scalar.dma_start(out=wk_t[m][:], in_=wk[m * 128:(m + 1) * 128, :])
    for k in range(KT):
        nc.scalar.dma_start(out=wv_sb[:, k, :], in_=wv[k * 128:(k + 1) * 128, :])

    # ---- persistent intermediates ----
    qt = [qkv.tile([128, NQ], F16, tag=f"qt{p}", name=f"qt{p}") for p in range(PAIRS)]
    kt = [qkv.tile([128, NK], F16, tag=f"kt{p}", name=f"kt{p}") for p in range(PAIRS)]
    vp_sb = qkv.tile([128, TKT, HPC, HD + 1], F16, tag="vp")  # V + ones col
    att = [qkv.tile([128, NQ], F16, tag=f"att{p}", name=f"att{p}") for p in range(PAIRS)]
    nc.vector.memset(vp_sb[:, :, :, HD:HD + 1], 1.0)

    scale = 1.0 / float(np.sqrt(HD))
    ex = {}

    def emit_quad(p, c, tkm):
        """One scoresT quad (2 heads x 128 keys x 512 queries) + fused exp."""
        ps = psum.tile([128, 2, 512], F32, tag="quad", bufs=2,
                       name=f"qps_{p}_{c}_{tkm}")
        et = expool.tile([128, 2, 512], F16, tag="ex", name=f"ex_{p}_{c}_{tkm}")
        for hh in range(2):
            r0 = hh * 64
            for mh in range(2):
                c0 = mh * 64
                nc.tensor.matmul(
                    out=ps[c0:c0 + 64, hh, :],
                    lhsT=kt[p][r0:r0 + 64, tkm * 128 + c0:tkm * 128 + c0 + 64],
                    rhs=qt[p][r0:r0 + 64, c * 512:(c + 1) * 512],
                    start=True, stop=True,
                    tile_position=(r0, c0))
        nc.scalar.activation(out=et[:], in_=ps[:],
                             func=mybir.ActivationFunctionType.Exp,
                             scale=scale, bias=zb[:])
        ex[(p, c, tkm)] = et

    # early quads: all of chunk 0 plus pair 0 of chunk 1, interleaved into the
    # projection chains so the exp stream runs nonstop from ~22us; chunk-0
    # pairs first so each PV iteration's exps complete as early as possible
    early = [(p, 0, k) for p in range(PAIRS) for k in range(TKT)]
    early += [(0, 1, k) for k in range(TKT)]
    early.sort(key=lambda t: (t[1], t[0]))
    ei = 0

    def pop_quads(n, ready_pairs):
        nonlocal ei
        while n > 0 and ei < len(early):
            p, c, k = early[ei]
            if p >= ready_pairs:
                return
            emit_quad(p, c, k)
            ei += 1
            n -= 1

    # ---- QK projections: 2-chain groups (both 512-query chunks of one
    # tensor) sharing LDWEIGHTS, k-outer so the PE tracks the DMA frontier.
    # Group order (0,q),(1,q),(0,k),(1,k),(2,q),(2,k),(3,q),(3,k): pair 1's
    # q-chains (xq + ungated wq1 only) fill pair 0's xk DMA window with real
    # work, so the PE never idles long enough to re-throttle.
    ready = 0
    for m, gi in ((0, 0), (1, 0), (0, 1), (1, 1),
                  (2, 0), (2, 1), (3, 0), (3, 1)):
        x_t, w_sb, dst = ((xq_t, wq_t[m], qt[m]) if gi == 0 else
                          (xk_t, wk_t[m], kt[m]))
        # quads of completed pairs between chain groups keep the exp
        # stream fed while this pair's projections accumulate
        pop_quads(2 if ready else 0, ready_pairs=ready)
        pp = [psum.tile([128, 512], F32, tag="ps", bufs=2,
                        name=f"ps_p{m}_{gi}_{n}") for n in range(NCH)]
        for k in range(KT):
            if k == 4:
                pop_quads(1, ready_pairs=ready)
            for n in range(NCH):
                nc.tensor.matmul(out=pp[n][:],
                                 lhsT=w_sb[:, k, :],
                                 rhs=x_t[k][:, n * 512:(n + 1) * 512],
                                 start=(k == 0), stop=(k == KT - 1))
        for n in range(NCH):
            nc.vector.tensor_copy(out=dst[:, n * 512:(n + 1) * 512],
                                  in_=pp[n][:])
        if gi == 1:
            # pair m's qt/kt both complete: its quads become poppable
            ready = m + 1
            pop_quads(2, ready_pairs=ready)

    # xv DMAs (sync queue): the tiles reuse the xq ring slots, released as
    # pair 3's q-chains finish reading each k-tile
    for k in range(KT):
        xv_t.append(xpool.tile([128, NK], F16, tag="x", name=f"xv{k}"))
        nc.sync.dma_start(out=xv_t[k][:], in_=xvT[k * 128:(k + 1) * 128, :])
    for p in range(PAIRS):
        nc.sync.dma_start(out=wo_sb[:, p, :], in_=wo[p * 128:(p + 1) * 128, :])

    # ---- V projection: 4 groups of 2 token-chains on the 2-slot ps ring ----
    for g in range(4):
        chains = []
        for t in range(g * 2, g * 2 + 2):
            psv = psum.tile([128, 512], F32, tag="ps", bufs=2, name=f"psv_{t}")
            chains.append((psv, t))
        for k in range(KT):
            if k == 4:
                pop_quads(1, ready_pairs=PAIRS)
            for psv, t in chains:
                nc.tensor.matmul(out=psv[:],
                                 lhsT=xv_t[k][:, t * 128:(t + 1) * 128],
                                 rhs=wv_sb[:, k, :], start=(k == 0),
                                 stop=(k == KT - 1))
        for psv, t in chains:
            nc.vector.tensor_copy(
                out=vp_sb[:, t, :, 0:HD],
                in_=psv[:].rearrange("p (h d) -> p h d", h=HPC))
            pop_quads(1, ready_pairs=PAIRS)
    # the remaining ~5 early quads fire right after PV_0 (see the seq loop):
    # they fill the exp stream while the PE transitions into the PV phase

    def norm_dve(p, c, pvq):
        """DVE half of the softmax normalization: PV copies (releasing the
        pvq banks), denominator row, reciprocal, f16 cast."""
        pvsb = nrm.tile([65, 2, 512], F16, tag="pvsb", name=f"pvsb_{p}_{c}")
        nc.vector.tensor_copy(out=pvsb[:, 0, :], in_=pvq[0:65, 0, :])
        nc.vector.tensor_copy(out=pvsb[:, 1, :], in_=pvq[0:65, 1, :])
        den2 = nrm.tile([1, 1024], F32, tag="den2", name=f"den2_{p}_{c}")
        nc.vector.tensor_copy(
            out=den2[:].rearrange("p (h q) -> p h q", h=2),
            in_=pvsb[64:65, :, :])
        rec2 = nrm.tile([1, 1024], F32, tag="rec2", name=f"rec2_{p}_{c}")
        nc.vector.reciprocal_approx_accurate(out=rec2[:], in_=den2[:],
                                             scratch=rscr2[:])
        rec2h = nrm.tile([1, 1024], F16, tag="rec2h", name=f"rec2h_{p}_{c}")
        nc.vector.tensor_copy(out=rec2h[:], in_=rec2[:])
        return (p, c, pvsb, rec2h)

    def norm_pe(pend):
        """PE half, applied one iteration later so the broadcast matmuls
        never wait on the DVE chain at the head of the PE queue: two
        concurrent col-tiled K=1 broadcasts, then the normalizing muls."""
        p, c, pvsb, rec2h = pend
        cs = slice(c * 512, (c + 1) * 512)
        rb = psum.tile([128, 512], F32, tag="ps", bufs=2, name=f"rb_{p}_{c}")
        nc.tensor.matmul(out=rb[0:64, :], lhsT=selA[:], rhs=rec2h[:, 0:512],
                         start=True, stop=True, tile_position=(0, 0))
        nc.tensor.matmul(out=rb[64:128, :], lhsT=selA[:], rhs=rec2h[:, 512:1024],
                         start=True, stop=True, tile_position=(0, 64))
        nc.vector.tensor_mul(out=att[p][0:64, cs], in0=pvsb[0:64, 0, :],
                             in1=rb[0:64, :])
        nc.vector.tensor_mul(out=att[p][64:128, cs], in0=pvsb[0:64, 1, :],
                             in1=rb[64:128, :])

    tiles_w = [(4 + qi, half) for qi in range(4) for half in range(2)]
    psos_w = []

    def emit_out_wide_pre():
        """Final-chunk out-projection, phase 1: ranks p4=0..2 for 4 output
        tiles in the quad pool — emitted BEFORE the last normalization so
        they aren't queued behind its broadcast matmuls."""
        for j in range(2):
            qa = psum.tile([128, 2, 512], F32, tag="quad", bufs=2,
                           name=f"oq_{j}")
            psos_w.extend([qa[:, 0, :], qa[:, 1, :]])
        for p4 in range(PAIRS - 1):
            for ti in range(4):
                q, half = tiles_w[ti]
                nc.tensor.matmul(out=psos_w[ti],
                                 lhsT=att[p4][:, q * 128:(q + 1) * 128],
                                 rhs=wo_sb[:, p4, half * 512:(half + 1) * 512],
                                 start=(p4 == 0), stop=False)

    def emit_out_wide_post():
        """Phase 2 (after the last norm's allocations): the pv slot + 2
        ps-ring tiles for tiles 4..7, then rank 3 everywhere, then copies."""
        ov = psum.tile([128, 2, 512], F32, tag="pv", bufs=1, name="ov")
        psos_w.extend([ov[:, 0, :], ov[:, 1, :]])
        for j in range(2):
            psos_w.append(psum.tile([128, 512], F32, tag="ps", bufs=2,
                                    name=f"os_{j}"))
        for p4 in range(PAIRS - 1):
            for ti in (4, 5, 6, 7):
                q, half = tiles_w[ti]
                nc.tensor.matmul(out=psos_w[ti],
                                 lhsT=att[p4][:, q * 128:(q + 1) * 128],
                                 rhs=wo_sb[:, p4, half * 512:(half + 1) * 512],
                                 start=(p4 == 0), stop=False)
        for ti in range(8):
            q, half = tiles_w[ti]
            nc.tensor.matmul(out=psos_w[ti],
                             lhsT=att[PAIRS - 1][:, q * 128:(q + 1) * 128],
                             rhs=wo_sb[:, PAIRS - 1, half * 512:(half + 1) * 512],
                             start=False, stop=True)
        for i, ((q, half), pso) in enumerate(zip(tiles_w, psos_w)):
            ot = ost.tile([128, 512], F16, tag="ot", name=f"ot_{q}_{half}")
            # exp stream is drained here: split copies scalar/vector
            if i % 2 == 0:
                nc.scalar.copy(out=ot[:], in_=pso)
            else:
                nc.vector.tensor_copy(out=ot[:], in_=pso)
            eng = nc.sync if i % 2 == 0 else nc.gpsimd
            eng.dma_start(out=out[q * 128:(q + 1) * 128,
                                  half * 512:(half + 1) * 512], in_=ot[:])

    tiles0 = [(qi, half) for qi in range(4) for half in range(2)]

    def emit_out0_part(ti_list):
        """2-3 chunk-0 out-projection tiles on the ps ring; spread across
        iterations 4-6 so their ring stalls hide in the exp-paced slack."""
        for j, ti in enumerate(ti_list):
            q, half = tiles0[ti]
            pso = psum.tile([128, 512], F32, tag="ps", bufs=2,
                            name=f"pso_{q}_{half}")
            for p4 in range(PAIRS):
                nc.tensor.matmul(out=pso[:],
                                 lhsT=att[p4][:, q * 128:(q + 1) * 128],
                                 rhs=wo_sb[:, p4, half * 512:(half + 1) * 512],
                                 start=(p4 == 0), stop=(p4 == PAIRS - 1))
            ot = ost.tile([128, 512], F16, tag="ot", name=f"ot_{q}_{half}")
            nc.vector.tensor_copy(out=ot[:], in_=pso[:])
            eng = nc.sync if ti % 2 == 0 else nc.gpsimd
            eng.dma_start(out=out[q * 128:(q + 1) * 128,
                                  half * 512:(half + 1) * 512], in_=ot[:])

    # ---- pipelined (pair, chunk) iterations ----
    # PV pairs write a dedicated 2-bank PSUM tile. Emission order per
    # iteration: PV, the PREVIOUS iteration's norm-PE half (its DVE inputs
    # are ready by now, so the broadcast matmuls never stall the PE queue),
    # quads for seq[i+5], out-proj blocks, then this iteration's norm-DVE.
    seq = [(p, c) for c in range(NCH) for p in range(PAIRS)]
    pending = None
    for i, (p, c) in enumerate(seq):
        pvq = psum.tile([128, 2, 512], F32, tag="pv", bufs=1,
                        name=f"pv_{p}_{c}")
        for k in range(TKT):
            et = ex[(p, c, k)]
            for hh in range(2):
                h = p * 2 + hh
                nc.tensor.matmul(out=pvq[0:65, hh, :],
                                 lhsT=vp_sb[:, k, h, :],
                                 rhs=et[:, hh, :],
                                 start=(k == 0), stop=(k == TKT - 1))
            del ex[(p, c, k)]
        if i == 0:
            pop_quads(len(early), ready_pairs=PAIRS)
        if pending is not None:
            norm_pe(pending)
        if i + 5 < len(seq):
            for k in range(TKT):
                emit_quad(seq[i + 5][0], seq[i + 5][1], k)
        if i == 4:
            emit_out0_part([0, 1, 2])
        elif i == 5:
            emit_out0_part([3, 4, 5])
        elif i == 6:
            emit_out0_part([6, 7])
        if i == len(seq) - 1:
            emit_out_wide_pre()
        pending = norm_dve(p, c, pvq)
    norm_pe(pending)
    emit_out_wide_post()
    ctx.close()


_NC_CACHE = None


def build():
    global _NC_CACHE
    if _NC_CACHE is None:
        nc = bacc.Bacc("TRN2", target_bir_lowering=False, debug=False,
                       num_devices=NCORES)
        with tile.TileContext(nc) as tc:
            _emit(tc)
        nc.compile()
        _NC_CACHE = nc
    return _NC_CACHE


def make_in_maps(inputs):
    q = np.asarray(inputs["query_tokens"], dtype=np.float32)
    kk = np.asarray(inputs["key_tokens"], dtype=np.float32)
    v = np.asarray(inputs["value_tokens"], dtype=np.float32)
    Wq = np.asarray(inputs["Wq"], dtype=np.float32)
    Wk = np.asarray(inputs["Wk"], dtype=np.float32)
    Wv = np.asarray(inputs["Wv"], dtype=np.float32)
    Wo = np.asarray(inputs["Wo"], dtype=np.float32)

    def swizzle(w_cols):
        # [1024, 512] -> [pair m][sbuf partition p][k-tile k][d]: block m is a
        # contiguous [128, 1024] so one DMA per pair lands pair-major
        a = w_cols.reshape(8, 128, 4, 128).transpose(2, 1, 0, 3)
        return np.ascontiguousarray(a.reshape(512, 1024)).astype(np.float16)

    qT = [np.ascontiguousarray(q[b].T).astype(np.float16) for b in range(B)]
    kT = [np.ascontiguousarray(kk[b].T).astype(np.float16) for b in range(B)]
    vT = [np.ascontiguousarray(v[b].T).astype(np.float16) for b in range(B)]
    wq_g = [swizzle(Wq[:, g * F:(g + 1) * F]) for g in range(2)]
    wk_g = [swizzle(Wk[:, g * F:(g + 1) * F]) for g in range(2)]
    wv_g = [np.ascontiguousarray(Wv[:, g * F:(g + 1) * F]).astype(np.float16)
            for g in range(2)]
    wo_g = [np.ascontiguousarray(Wo[g * F:(g + 1) * F, :]).astype(np.float16)
            for g in range(2)]

    in_maps = []
    for c in range(NCORES):
        b, g = c // 2, c % 2
        in_maps.append({
            "xqT": qT[b], "xkT": kT[b], "xvT": vT[b],
            "wq": wq_g[g], "wk": wk_g[g], "wv": wv_g[g], "wo": wo_g[g],
        })
    return in_maps


def combine(results, bo):
    out = np.zeros((B, NQ, D), dtype=np.float32)
    for c in range(NCORES):
        out[c // 2] += results[c]["out"].astype(np.float32)
    out += np.asarray(bo, dtype=np.float32)[None, None, :]
    return out


def kernel(**inputs):
    nc = build()
    in_maps = make_in_maps(inputs)
    res = run_bass_kernel_spmd(nc, in_maps, list(range(NCORES)))
    return combine(res.results, inputs["bo"])


# revision 42
# speedup vs baseline: 1.2334x; 1.0328x over previous
"""Trainium2 Bass kernel for nn_CrossAttention (B=4, NQ=NK=1024, D=1024, H=16).

Sharding: 8 cores = 4 batches x 2 head-groups (8 heads each). Per core:
  - inputs arrive pre-transposed/sliced on host (free): xqT/xkT/xvT [D, T] fp16,
    Wq/Wk/Wv column slices [D, 512] fp16, Wo row slice [512, D] fp16.
  - warm-up matmuls on a zeroed scratch tile run from t~7us (during the DMA
    fill) so the PE HAM clock-gate is released before real matmuls start.
  - input DMAs spread over 4 hw queues (gpsimd/sync/vector/scalar) with pair-0
    weights at the head of the gpsimd queue and xq/xk each split even/odd
    across two queues, so the projection k-frontier is never DMA-starved.
  - projections produce Q^T/K^T per head-pair [128, T] (lhsT = W slice, rhs =
    xT) in 2-chain groups (shared LDWEIGHTS) on a 2-slot PSUM ring.
  - scores computed transposed (scoresT [Tk, Tq]) as 4-way tile_position-packed
    quads (2 heads x 2 M-halves, K=64 each) into 2-bank PSUM tiles (3-deep
    ring); ONE fused exp activation per quad (free size 1024).
  - the exp stream starts at ~21us and must never starve: 40 early quads
    (all of chunk 0 + pair 0 of chunk 1) are interleaved into the projection
    chains, the rest follow with +5 lookahead in the PV loop.
  - denominators via an augmented ones-column in V (row 64 of the PV output);
    reciprocal via 2-op approx on [2,512]; the per-query reciprocals are
    broadcast to 128 partitions with ONE K=2 PE matmul (selector lhsT), so
    gpsimd is off the critical path entirely.
  - per query-chunk out-projection overlaps the next chunk's scores/PV; final
    chunk accumulates p4-major across 8 PSUM banks, copies alternate
    scalar/vector, output DMAs alternate sync/gpsimd.
  - host sums the two head-group partials per batch and adds the bias.
All matmuls fp16 (1 cycle/row on PE), accumulation fp32 in PSUM.
"""
import sys

sys.path.insert(0, "/opt/trn_rl_repo")

from contextlib import ExitStack

import numpy as np

import concourse.bass as bass
import concourse.tile as tile
from concourse import bacc, mybir
from concourse.bass_utils import run_bass_kernel_spmd

F32 = mybir.dt.float32
F16 = mybir.dt.float16

B, NQ, NK, D, H, HD = 4, 1024, 1024, 1024, 16, 64
NCORES = 8
HPC = 8          # heads per core
F = HPC * HD     # 512: per-core projection width
KT = D // 128    # 8 k-tiles over D
PAIRS = HPC // 2  # 4 head pairs
TKT = NK // 128  # 8 tiles over key tokens
NCH = NQ // 512  # 2 query chunks

N_WARM = 8       # warm-up matmuls to release the HAM throttle
H1_DIRECT = True  # DVE mul writing partitions 64-127 from inputs at 0-63


def _emit(tc):
    nc = tc.nc
    ctx = ExitStack()

    xqT = nc.dram_tensor("xqT", [D, NQ], F16, kind="ExternalInput").ap()
    xkT = nc.dram_tensor("xkT", [D, NK], F16, kind="ExternalInput").ap()
    xvT = nc.dram_tensor("xvT", [D, NK], F16, kind="ExternalInput").ap()
    # wq/wk host-swizzled to [m-pair][sbuf-partition][k-tile*128]: one
    # contiguous 256KB DMA per pair
    wq = nc.dram_tensor("wq", [PAIRS * 128, D], F16, kind="ExternalInput").ap()
    wk = nc.dram_tensor("wk", [PAIRS * 128, D], F16, kind="ExternalInput").ap()
    wv = nc.dram_tensor("wv", [D, F], F16, kind="ExternalInput").ap()
    wo = nc.dram_tensor("wo", [F, D], F16, kind="ExternalInput").ap()
    out = nc.dram_tensor("out", [NQ, D], F16, kind="ExternalOutput").ap()

    wpool = ctx.enter_context(tc.tile_pool(name="wpool", bufs=1))
    qkv = ctx.enter_context(tc.tile_pool(name="qkv", bufs=1))
    xpool = ctx.enter_context(tc.tile_pool(name="xpool", bufs=16))
    expool = ctx.enter_context(tc.tile_pool(name="expool", bufs=42))
    psum = ctx.enter_context(tc.tile_pool(name="psum", bufs=1, space="PSUM"))
    nrm = ctx.enter_context(tc.tile_pool(name="nrm", bufs=2))
    ost = ctx.enter_context(tc.tile_pool(name="ost", bufs=4))

    # zero-bias AP for activations (avoids the const-page TENSOR_LOAD)
    zb = nrm.tile([128, 1], F32, tag="zb", bufs=1)
    nc.vector.memset(zb[:], 0.0)
    # ones selector for the K=1 reciprocal-broadcast matmuls
    selA = nrm.tile([1, 64], F16, tag="selA", bufs=1)
    nc.vector.memset(selA[:], 1.0)
    # warm-up scratch (memset on gpsimd: ready before the vector preamble)
    warm = nrm.tile([128, 512], F16, tag="warm", bufs=1)
    nc.gpsimd.memset(warm[:], 0.0)
    rscr2 = nrm.tile([1, 1024], F32, tag="rscr2", bufs=1)
    # preload the exp table set during the DMA fill (saves ~2.7us at the
    # first real exp)
    tload = nrm.tile([128, 1], F16, tag="tload", bufs=1)
    nc.scalar.activation(out=tload[:], in_=zb[:],
                         func=mybir.ActivationFunctionType.Exp,
                         scale=1.0, bias=zb[:])

    # ---- warm-up matmuls: PE busy from t~7.3us so HAM hits K=8/8 by the time
    # the first real projection matmul runs
    for i in range(N_WARM):
        wps = psum.tile([128, 512], F32, tag="ps", bufs=2, name=f"warm{i}")
        nc.tensor.matmul(out=wps[:], lhsT=warm[:, 0:128], rhs=warm[:],
                         start=True, stop=True)

    # ---- input DMAs over 3 queues. x tiles get the gpsimd AND sync queues
    # (even/odd split per tensor); weights ride the scalar queue with pairs
    # 2/3 in a 2-slot ring so their transfers self-gate (WAR semaphore)
    # behind pair 0/1's projection reads instead of crowding the x stream.
    wq_t = [wpool.tile([128, KT, 128], F16, tag="wqr", bufs=2, name=f"wqr{m}")
            for m in range(PAIRS)]
    wk_t = [wpool.tile([128, KT, 128], F16, tag="wkr", bufs=2, name=f"wkr{m}")
            for m in range(PAIRS)]
    wv_sb = wpool.tile([128, KT, F], F16, tag="wv")
    wo_sb = wpool.tile([128, PAIRS, D], F16, tag="wo")

    xq_t, xk_t, xv_t = [], [], []
    for k in range(KT):
        xq_t.append(xpool.tile([128, NQ], F16, tag="x", name=f"xq{k}"))
        xk_t.append(xpool.tile([128, NK], F16, tag="x", name=f"xk{k}"))

    # The gpsimd software-DGE queue outcompetes the HW-DGE queues on the
    # shared DMA engines, so the ENTIRE phase-1 critical sequence rides it
    # in exact consumption order; wv (needed ~20us later) goes to sync.
    nc.gpsimd.dma_start(out=xq_t[0][:], in_=xqT[0:128, :])
    nc.gpsimd.dma_start(out=wq_t[0][:], in_=wq[0:128, :])
    nc.gpsimd.dma_start(out=xq_t[1][:], in_=xqT[128:256, :])
    nc.gpsimd.dma_start(out=wq_t[1][:], in_=wq[128:256, :])
    for k in range(2, KT):
        nc.gpsimd.dma_start(out=xq_t[k][:], in_=xqT[k * 128:(k + 1) * 128, :])
    nc.gpsimd.dma_start(out=wk_t[0][:], in_=wk[0:128, :])
    nc.gpsimd.dma_start(out=wk_t[1][:], in_=wk[128:256, :])
    for k in range(KT):
        nc.gpsimd.dma_start(out=xk_t[k][:], in_=xkT[k * 128:(k + 1) * 128, :])
    # scalar queue: the ring-gated pair 2/3 weights (their descriptors
    # self-gate via WAR behind pair 0/1's reads), then wv — so this queue
    # never competes with the phase-1 x stream
    for m in range(2, PAIRS):
        nc.scalar.dma_start(out=wq_t[m][:], in_=wq[m * 128:(m + 1) * 128, :])
        nc.scalar.dma_start(out=wk_t[m][:], in_=wk[m * 128:(m + 1) * 128, :])
    for k in range(KT):
        nc.scalar.dma_start(out=wv_sb[:, k, :], in_=wv[k * 128:(k + 1) * 128, :])

    # ---- persistent intermediates ----
    qt = [qkv.tile([128, NQ], F16, tag=f"qt{p}", name=f"qt{p}") for p in range(PAIRS)]
    kt = [qkv.tile([128, NK], F16, tag=f"kt{p}", name=f"kt{p}") for p in range(PAIRS)]
    vp_sb = qkv.tile([128, TKT, HPC, HD + 1], F16, tag="vp")  # V + ones col
    att = [qkv.tile([128, NQ], F16, tag=f"att{p}", name=f"att{p}") for p in range(PAIRS)]
    nc.vector.memset(vp_sb[:, :, :, HD:HD + 1], 1.0)

    scale = 1.0 / float(np.sqrt(HD))
    ex = {}

    def emit_quad(p, c, tkm):
        """One scoresT quad (2 heads x 128 keys x 512 queries) + fused exp."""
        ps = psum.tile([128, 2, 512], F32, tag="quad", bufs=2,
                       name=f"qps_{p}_{c}_{tkm}")
        et = expool.tile([128, 2, 512], F16, tag="ex", name=f"ex_{p}_{c}_{tkm}")
        for hh in range(2):
            r0 = hh * 64
            for mh in range(2):
                c0 = mh * 64
                nc.tensor.matmul(
                    out=ps[c0:c0 + 64, hh, :],
                    lhsT=kt[p][r0:r0 + 64, tkm * 128 + c0:tkm * 128 + c0 + 64],
                    rhs=qt[p][r0:r0 + 64, c * 512:(c + 1) * 512],
                    start=True, stop=True,
                    tile_position=(r0, c0))
        nc.scalar.activation(out=et[:], in_=ps[:],
                             func=mybir.ActivationFunctionType.Exp,
                             scale=scale, bias=zb[:])
        ex[(p, c, tkm)] = et

    # early quads: all of chunk 0 plus pair 0 of chunk 1, interleaved into the
    # projection chains so the exp stream runs nonstop from ~22us; chunk-0
    # pairs first so each PV iteration's exps complete as early as possible
    early = [(p, 0, k) for p in range(PAIRS) for k in range(TKT)]
    early += [(0, 1, k) for k in range(TKT)]
    early.sort(key=lambda t: (t[1], t[0]))
    ei = 0

    def pop_quads(n, ready_pairs):
        nonlocal ei
        while n > 0 and ei < len(early):
            p, c, k = early[ei]
            if p >= ready_pairs:
                return
            emit_quad(p, c, k)
            ei += 1
            n -= 1

    # ---- QK projections: 2-chain groups (both 512-query chunks of one
    # tensor) sharing LDWEIGHTS, k-outer so the PE tracks the DMA frontier.
    # Group order (0,q),(1,q),(0,k),(1,k),(2,q),(2,k),(3,q),(3,k): pair 1's
    # q-chains (xq + ungated wq1 only) fill pair 0's xk DMA window with real
    # work, so the PE never idles long enough to re-throttle.
    ready = 0
    for m, gi in ((0, 0), (1, 0), (0, 1), (1, 1),
                  (2, 0), (2, 1), (3, 0), (3, 1)):
        x_t, w_sb, dst = ((xq_t, wq_t[m], qt[m]) if gi == 0 else
                          (xk_t, wk_t[m], kt[m]))
        # quads of completed pairs between chain groups keep the exp
        # stream fed while this pair's projections accumulate
        pop_quads(2 if ready else 0, ready_pairs=ready)
        pp = [psum.tile([128, 512], F32, tag="ps", bufs=2,
                        name=f"ps_p{m}_{gi}_{n}") for n in range(NCH)]
        for k in range(KT):
            if k == 4:
                pop_quads(1, ready_pairs=ready)
            for n in range(NCH):
                nc.tensor.matmul(out=pp[n][:],
                                 lhsT=w_sb[:, k, :],
                                 rhs=x_t[k][:, n * 512:(n + 1) * 512],
                                 start=(k == 0), stop=(k == KT - 1))
        for n in range(NCH):
            nc.vector.tensor_copy(out=dst[:, n * 512:(n + 1) * 512],
                                  in_=pp[n][:])
        if gi == 1:
            # pair m's qt/kt both complete: its quads become poppable
            ready = m + 1
            pop_quads(2, ready_pairs=ready)

    # xv DMAs (sync queue): the tiles reuse the xq ring slots, released as
    # pair 3's q-chains finish reading each k-tile
    for k in range(KT):
        xv_t.append(xpool.tile([128, NK], F16, tag="x", name=f"xv{k}"))
        nc.sync.dma_start(out=xv_t[k][:], in_=xvT[k * 128:(k + 1) * 128, :])
    for p in range(PAIRS):
        nc.sync.dma_start(out=wo_sb[:, p, :], in_=wo[p * 128:(p + 1) * 128, :])

    # ---- V projection: 4 groups of 2 token-chains on the 2-slot ps ring ----
    for g in range(4):
        chains = []
        for t in range(g * 2, g * 2 + 2):
            psv = psum.tile([128, 512], F32, tag="ps", bufs=2, name=f"psv_{t}")
            chains.append((psv, t))
        for k in range(KT):
            if k == 4:
                pop_quads(1, ready_pairs=PAIRS)
            for psv, t in chains:
                nc.tensor.matmul(out=psv[:],
                                 lhsT=xv_t[k][:, t * 128:(t + 1) * 128],
                                 rhs=wv_sb[:, k, :], start=(k == 0),
                                 stop=(k == KT - 1))
        for psv, t in chains:
            nc.vector.tensor_copy(
                out=vp_sb[:, t, :, 0:HD],
                in_=psv[:].rearrange("p (h d) -> p h d", h=HPC))
            pop_quads(1, ready_pairs=PAIRS)
    # the remaining ~5 early quads fire right after PV_0 (see the seq loop):
    # they fill the exp stream while the PE transitions into the PV phase

    def norm_dve(p, c, pvq):
        """DVE half of the softmax normalization: PV copies (releasing the
        pvq banks), denominator row, reciprocal, f16 cast."""
        pvsb = nrm.tile([65, 2, 512], F16, tag="pvsb", name=f"pvsb_{p}_{c}")
        nc.vector.tensor_copy(out=pvsb[:, 0, :], in_=pvq[0:65, 0, :])
        nc.vector.tensor_copy(out=pvsb[:, 1, :], in_=pvq[0:65, 1, :])
        den2 = nrm.tile([1, 1024], F32, tag="den2", name=f"den2_{p}_{c}")
        nc.vector.tensor_copy(
            out=den2[:].rearrange("p (h q) -> p h q", h=2),
            in_=pvsb[64:65, :, :])
        rec2 = nrm.tile([1, 1024], F32, tag="rec2", name=f"rec2_{p}_{c}")
        nc.vector.reciprocal_approx_accurate(out=rec2[:], in_=den2[:],
                                             scratch=rscr2[:])
        rec2h = nrm.tile([1, 1024], F16, tag="rec2h", name=f"rec2h_{p}_{c}")
        nc.vector.tensor_copy(out=rec2h[:], in_=rec2[:])
        return (p, c, pvsb, rec2h)

    def norm_pe(pend):
        """PE half, applied one iteration later so the broadcast matmuls
        never wait on the DVE chain at the head of the PE queue: two
        concurrent col-tiled K=1 broadcasts, then the normalizing muls."""
        p, c, pvsb, rec2h = pend
        cs = slice(c * 512, (c + 1) * 512)
        rb = psum.tile([128, 512], F32, tag="ps", bufs=2, name=f"rb_{p}_{c}")
        nc.tensor.matmul(out=rb[0:64, :], lhsT=selA[:], rhs=rec2h[:, 0:512],
                         start=True, stop=True, tile_position=(0, 0))
        nc.tensor.matmul(out=rb[64:128, :], lhsT=selA[:], rhs=rec2h[:, 512:1024],
                         start=True, stop=True, tile_position=(0, 64))
        nc.vector.tensor_mul(out=att[p][0:64, cs], in0=pvsb[0:64, 0, :],
                             in1=rb[0:64, :])
        nc.vector.tensor_mul(out=att[p][64:128, cs], in0=pvsb[0:64, 1, :],
                             in1=rb[64:128, :])

    tiles_w = [(4 + qi, half) for qi in range(4) for half in range(2)]
    psos_w = []

    def emit_out_wide_pre():
        """Final-chunk out-projection, phase 1: ranks p4=0..2 for 4 output
        tiles in the quad pool — emitted BEFORE the last normalization so
        they aren't queued behind its broadcast matmuls."""
        for j in range(2):
            qa = psum.tile([128, 2, 512], F32, tag="quad", bufs=2,
                           name=f"oq_{j}")
            psos_w.extend([qa[:, 0, :], qa[:, 1, :]])
        for p4 in range(PAIRS - 1):
            for ti in range(4):
                q, half = tiles_w[ti]
                nc.tensor.matmul(out=psos_w[ti],
                                 lhsT=att[p4][:, q * 128:(q + 1) * 128],
                                 rhs=wo_sb[:, p4, half * 512:(half + 1) * 512],
                                 start=(p4 == 0), stop=False)

    def emit_out_wide_post():
        """Phase 2 (after the last norm's allocations): the pv slot + 2
        ps-ring tiles for tiles 4..7, then rank 3 everywhere, then copies."""
        ov = psum.tile([128, 2, 512], F32, tag="pv", bufs=1, name="ov")
        psos_w.extend([ov[:, 0, :], ov[:, 1, :]])
        for j in range(2):
            psos_w.append(psum.tile([128, 512], F32, tag="ps", bufs=2,
                                    name=f"os_{j}"))
        for p4 in range(PAIRS - 1):
            for ti in (4, 5, 6, 7):
                q, half = tiles_w[ti]
                nc.tensor.matmul(out=psos_w[ti],
                                 lhsT=att[p4][:, q * 128:(q + 1) * 128],
                                 rhs=wo_sb[:, p4, half * 512:(half + 1) * 512],
                                 start=(p4 == 0), stop=False)
        for ti in range(8):
            q, half = tiles_w[ti]
            nc.tensor.matmul(out=psos_w[ti],
                             lhsT=att[PAIRS - 1][:, q * 128:(q + 1) * 128],
                             rhs=wo_sb[:, PAIRS - 1, half * 512:(half + 1) * 512],
                             start=False, stop=True)
        for i, ((q, half), pso) in enumerate(zip(tiles_w, psos_w)):
            ot = ost.tile([128, 512], F16, tag="ot", name=f"ot_{q}_{half}")
            # exp stream is drained here: split copies scalar/vector
            if i % 2 == 0:
                nc.scalar.copy(out=ot[:], in_=pso)
            else:
                nc.vector.tensor_copy(out=ot[:], in_=pso)
            eng = nc.sync if i % 2 == 0 else nc.gpsimd
            eng.dma_start(out=out[q * 128:(q + 1) * 128,
                                  half * 512:(half + 1) * 512], in_=ot[:])

    tiles0 = [(qi, half) for qi in range(4) for half in range(2)]

    def emit_out0_part(ti_list):
        """2-3 chunk-0 out-projection tiles on the ps ring; spread across
        iterations 4-6 so their ring stalls hide in the exp-paced slack."""
        for j, ti in enumerate(ti_list):
            q, half = tiles0[ti]
            pso = psum.tile([128, 512], F32, tag="ps", bufs=2,
                            name=f"pso_{q}_{half}")
            for p4 in range(PAIRS):
                nc.tensor.matmul(out=pso[:],
                                 lhsT=att[p4][:, q * 128:(q + 1) * 128],
                                 rhs=wo_sb[:, p4, half * 512:(half + 1) * 512],
                                 start=(p4 == 0), stop=(p4 == PAIRS - 1))
            ot = ost.tile([128, 512], F16, tag="ot", name=f"ot_{q}_{half}")
            nc.vector.tensor_copy(out=ot[:], in_=pso[:])
            eng = nc.sync if ti % 2 == 0 else nc.gpsimd
            eng.dma_start(out=out[q * 128:(q + 1) * 128,
                                  half * 512:(half + 1) * 512], in_=ot[:])

    # ---- pipelined (pair, chunk) iterations ----
    # PV pairs write a dedicated 2-bank PSUM tile. Emission order per
    # iteration: PV, the PREVIOUS iteration's norm-PE half (its DVE inputs
    # are ready by now, so the broadcast matmuls never stall the PE queue),
    # quads for seq[i+5], out-proj blocks, then this iteration's norm-DVE.
    # late quad batches are placed explicitly: (1,1)/(2,1) right after
    # PV_0/PV_1 (ScalarE continuity), but (3,1) only after PV_5 — so PV_2..6,
    # the norms, and out0 run DURING the exp drain instead of queueing
    # behind ScalarE-paced quad blocks
    late_quads = {0: (1, 1), 1: (2, 1), 5: (3, 1)}
    out0_parts = {4: [0, 1, 2], 5: [3, 4, 5], 6: [6, 7]}
    seq = [(p, c) for c in range(NCH) for p in range(PAIRS)]
    pending = None
    for i, (p, c) in enumerate(seq):
        pvq = psum.tile([128, 2, 512], F32, tag="pv", bufs=1,
                        name=f"pv_{p}_{c}")
        for k in range(TKT):
            et = ex[(p, c, k)]
            for hh in range(2):
                h = p * 2 + hh
                nc.tensor.matmul(out=pvq[0:65, hh, :],
                                 lhsT=vp_sb[:, k, h, :],
                                 rhs=et[:, hh, :],
                                 start=(k == 0), stop=(k == TKT - 1))
            del ex[(p, c, k)]
        if i == 0:
            pop_quads(len(early), ready_pairs=PAIRS)
        if pending is not None:
            norm_pe(pending)
        if i in out0_parts:
            emit_out0_part(out0_parts[i])
        if i in late_quads:
            lp, lc = late_quads[i]
            for k in range(TKT):
                emit_quad(lp, lc, k)
        if i == len(seq) - 1:
            emit_out_wide_pre()
        pending = norm_dve(p, c, pvq)
    norm_pe(pending)
    emit_out_wide_post()
    ctx.close()


_NC_CACHE = None


def build():
    global _NC_CACHE
    if _NC_CACHE is None:
        nc = bacc.Bacc("TRN2", target_bir_lowering=False, debug=False,
                       num_devices=NCORES)
        with tile.TileContext(nc) as tc:
            _emit(tc)
        nc.compile()
        _NC_CACHE = nc
    return _NC_CACHE


def make_in_maps(inputs):
    q = np.asarray(inputs["query_tokens"], dtype=np.float32)
    kk = np.asarray(inputs["key_tokens"], dtype=np.float32)
    v = np.asarray(inputs["value_tokens"], dtype=np.float32)
    Wq = np.asarray(inputs["Wq"], dtype=np.float32)
    Wk = np.asarray(inputs["Wk"], dtype=np.float32)
    Wv = np.asarray(inputs["Wv"], dtype=np.float32)
    Wo = np.asarray(inputs["Wo"], dtype=np.float32)

    def swizzle(w_cols):
        # [1024, 512] -> [pair m][sbuf partition p][k-tile k][d]: block m is a
        # contiguous [128, 1024] so one DMA per pair lands pair-major
        a = w_cols.reshape(8, 128, 4, 128).transpose(2, 1, 0, 3)
        return np.ascontiguousarray(a.reshape(512, 1024)).astype(np.float16)

    qT = [np.ascontiguousarray(q[b].T).astype(np.float16) for b in range(B)]
    kT = [np.ascontiguousarray(kk[b].T).astype(np.float16) for b in range(B)]
    vT = [np.ascontiguousarray(v[b].T).astype(np.float16) for b in range(B)]
    wq_g = [swizzle(Wq[:, g * F:(g + 1) * F]) for g in range(2)]
    wk_g = [swizzle(Wk[:, g * F:(g + 1) * F]) for g in range(2)]
    wv_g = [np.ascontiguousarray(Wv[:, g * F:(g + 1) * F]).astype(np.float16)
            for g in range(2)]
    wo_g = [np.ascontiguousarray(Wo[g * F:(g + 1) * F, :]).astype(np.float16)
            for g in range(2)]

    in_maps = []
    for c in range(NCORES):
        b, g = c // 2, c % 2
        in_maps.append({
            "xqT": qT[b], "xkT": kT[b], "xvT": vT[b],
            "wq": wq_g[g], "wk": wk_g[g], "wv": wv_g[g], "wo": wo_g[g],
        })
    return in_maps


def combine(results, bo):
    out = np.zeros((B, NQ, D), dtype=np.float32)
    for c in range(NCORES):
        out[c // 2] += results[c]["out"].astype(np.float32)
    out += np.asarray(bo, dtype=np.float32)[None, None, :]
    return out


def kernel(**inputs):
    nc = build()
    in_maps = make_in_maps(inputs)
    res = run_bass_kernel_spmd(nc, in_maps, list(range(NCORES)))
    return combine(res.results, inputs["bo"])
